# revision 1
# baseline (speedup 1.0000x reference)
"""Trainium2 Bass kernel for the gnn_message_passing problem.

Strategy (8 NeuronCores, SPMD), v2:
  - Host computes U = unique(inputs|item) (~32k of 50k vocab rows actually
    referenced).  Each core projects a 4096-row slice of U to the 128-dim
    item embedding (emb + 0.1*(img@Wi+bi) + 0.15*(txt@Wt+bt)) using
    weights-stationary N=512 bf16 matmuls in transposed orientation
    ([dout, rows]), then PE-transposes back to row-major bf16.
  - ONE bf16 AllGather of the compact item table (8.4MB vs 51MB in v1).
    The Tvis (img|txt projected) all-gather is eliminated entirely via
    linearity: session_img = (W^T @ masked_row_sum + b*cnt)/denom, so the
    session means are computed from RAW table rows gathered per batch
    shard from a U-compacted [32769, 1896] bf16 concat table, with
    mask-stationary [100,2]x[100,512] matmuls (few, large instructions).
  - Batch-sharded phase C: session fusion math in transposed [128, 64]
    layout (as v1), hypergraph layers per session pair in bf16.
"""

import sys

sys.path.insert(0, "/opt/trn_rl_repo")

import numpy as np
import ml_dtypes

import concourse.bass as bass
import concourse.bacc as bacc
import concourse.mybir as mybir
import concourse.tile as tile
from concourse import bass_utils

BF16 = ml_dtypes.bfloat16


class Cfg:
    def __init__(self):
        self.N = 50000
        self.D = 128
        self.IMG = 1000
        self.TXT = 768
        self.B = 512
        self.L = 50
        self.NC = 8
        self.UC = 4096                 # U rows projected per core
        self.NU = self.NC * self.UC    # 32768 capacity
        self.NF = 1 + self.NU          # padded table rows (row 0 = zeros)
        self.BS = self.B // self.NC    # 64 sessions per core
        self.NPAIR = self.BS // 2      # 32
        self.L2 = 2 * self.L           # 100
        self.KI = 8                    # img k-chunks of 125
        self.KIW = 125
        self.KT = 6                    # txt k-chunks of 128
        self.KTW = 128
        self.RAWW = self.IMG + self.TXT + self.D   # 1896
        self.ACH = self.UC // 512      # 8 phase-A chunks of 512 rows


REAL = Cfg()


def build_program(c: Cfg):
    f32 = mybir.dt.float32
    b16 = mybir.dt.bfloat16
    i32 = mybir.dt.int32
    AF = mybir.ActivationFunctionType
    AX = mybir.AxisListType
    OP = mybir.AluOpType

    nc = bacc.Bacc("TRN2", target_bir_lowering=False, debug=False,
                   num_devices=c.NC)

    def ein(nm, sh, dt):
        return nc.dram_tensor(nm, sh, dt, kind="ExternalInput")

    imgRT = ein("imgRT", [c.KIW, c.ACH, c.KI, 512], b16)  # [f, cc, k, v]
    txtRT = ein("txtRT", [c.KTW, c.ACH, c.KT, 512], b16)
    embRT = ein("embRT", [c.D, c.ACH, 512], f32)          # bias baked in
    wiN = ein("wiN", [c.IMG, c.D], b16)         # img_W
    wtN = ein("wtN", [c.TXT, c.D], b16)
    wi01 = ein("wi01", [c.IMG, c.D], b16)       # 0.1 * img_W
    wt015 = ein("wt015", [c.TXT, c.D], b16)     # 0.15 * txt_W
    imgbR = ein("imgbR", [1, c.D], b16)
    txtbR = ein("txtbR", [1, c.D], b16)
    rawcat = ein("rawcat", [c.NF, c.RAWW], mybir.dt.float8e4)
    gvW = ein("gvW", [c.D, c.D], f32)
    gvB = ein("gvB", [c.D, 1], f32)
    gtW = ein("gtW", [c.D, c.D], f32)
    gtB = ein("gtB", [c.D, 1], f32)
    q1W = ein("q1W", [c.D, c.D], f32)
    q1B = ein("q1B", [c.D, 1], f32)
    q2W = ein("q2W", [c.D, 1], f32)
    Gbd = ein("Gbd", [c.NPAIR, c.L2, c.L2], b16)
    GTbd = ein("GTbd", [c.NPAIR, c.L2, c.L2], b16)
    Mbd16 = ein("Mbd16", [c.NPAIR, c.L2, 2], mybir.dt.float8e4)
    mkT = ein("mkT", [c.L, c.BS], b16)          # mask.T
    mindT = ein("mindT", [c.L, c.BS], b16)      # (mask * (item>0)).T
    ind2 = ein("ind2", [2, c.L2], b16)
    h0idx = ein("h0idx", [c.NPAIR, c.L2, 1], i32)
    ssidx = ein("ssidx", [c.NPAIR, c.L2, 1], i32)

    outH = nc.dram_tensor("outH", [c.BS, c.L, c.D], f32, kind="ExternalOutput")

    localI = nc.dram_tensor("localI", [c.UC, c.D], b16)
    XsDram = nc.dram_tensor("XsDram", [c.BS, c.D], b16)
    Titem = nc.dram_tensor("Titem", [c.NF, c.D], b16, addr_space="Shared")

    rg = [list(range(c.NC))]
    # raw-concat column chunks for transposes / projection
    # img: 8 x 125, txt: 6 x 128, emb: 1 x 128
    CH = [(k * c.KIW, c.KIW) for k in range(c.KI)]
    CH += [(c.IMG + k * c.KTW, c.KTW) for k in range(c.KT)]
    CH += [(c.IMG + c.TXT, c.D)]
    # session-sum segments (psum free-dim <= 512)
    SEG = [(0, 512), (512, 512), (1024, 512), (1536, 360)]

    with tile.TileContext(nc) as tc:
        with (
            tc.tile_pool(name="wpool", bufs=1) as wp,
            tc.tile_pool(name="apool", bufs=3) as ap,
            tc.tile_pool(name="ostg", bufs=2) as ost,
            tc.tile_pool(name="cbig", bufs=1) as cb,
            tc.tile_pool(name="cgat", bufs=2) as cg,
            tc.tile_pool(name="csml", bufs=3) as cs,
        ):
            # ---- weights / constants ----
            wi01t = [wp.tile([c.KIW, c.D], b16, tag=f"wi01_{k}", name=f"wi01_{k}")
                     for k in range(c.KI)]
            wt015t = [wp.tile([c.KTW, c.D], b16, tag=f"wt015_{k}", name=f"wt015_{k}")
                      for k in range(c.KT)]
            wiNt = [wp.tile([c.KIW, c.D], b16, tag=f"wiN_{k}", name=f"wiN_{k}")
                    for k in range(c.KI)]
            wtNt = [wp.tile([c.KTW, c.D], b16, tag=f"wtN_{k}", name=f"wtN_{k}")
                    for k in range(c.KT)]
            for k in range(c.KI):
                nc.sync.dma_start(wi01t[k][:], wi01[k * c.KIW:(k + 1) * c.KIW, :])
                nc.sync.dma_start(wiNt[k][:], wiN[k * c.KIW:(k + 1) * c.KIW, :])
            for k in range(c.KT):
                nc.sync.dma_start(wt015t[k][:], wt015[k * c.KTW:(k + 1) * c.KTW, :])
                nc.sync.dma_start(wtNt[k][:], wtN[k * c.KTW:(k + 1) * c.KTW, :])
            bir_ = wp.tile([1, c.D], b16, tag="bir")
            btr = wp.tile([1, c.D], b16, tag="btr")
            nc.sync.dma_start(bir_[:], imgbR[:])
            nc.sync.dma_start(btr[:], txtbR[:])
            ident16 = wp.tile([128, 128], b16, tag="id16")
            identf = wp.tile([128, 128], f32, tag="idf")
            from concourse.masks import make_identity
            make_identity(nc, ident16[:])
            make_identity(nc, identf[:])

            # zero row 0 of Titem
            zi = wp.tile([1, c.D], b16, tag="zi")
            nc.vector.memset(zi[:], 0.0)
            nc.sync.dma_start(Titem[0:1, :], zi[:])

            # phase-C persistent loads (start DMAs early; no dep on phase A)
            m16 = cb.tile([c.L2, c.NPAIR * 2], mybir.dt.float8e4, tag="m16")
            nc.sync.dma_start(
                m16[:].rearrange("l (p j) -> l p j", p=c.NPAIR),
                Mbd16.rearrange("p l j -> l p j"))
            hix = cb.tile([c.L2, c.NPAIR], i32, tag="hix")
            six = cb.tile([c.L2, c.NPAIR], i32, tag="six")
            nc.sync.dma_start(hix[:], h0idx.rearrange("p l o -> l (p o)"))
            nc.sync.dma_start(six[:], ssidx.rearrange("p l o -> l (p o)"))
            mkTt = cb.tile([c.L, c.BS], b16, tag="mkT")
            minTt = cb.tile([c.L, c.BS], b16, tag="minT")
            nc.sync.dma_start(mkTt[:], mkT[:])
            nc.sync.dma_start(minTt[:], mindT[:])
            i2 = cb.tile([2, c.L2], b16, tag="i2")
            nc.sync.dma_start(i2[:], ind2[:])
            ones50 = cb.tile([c.L, 1], b16, tag="ones50")
            nc.vector.memset(ones50[:], 1.0)
            wgv = cb.tile([c.D, c.D], f32, tag="wgv")
            wgt = cb.tile([c.D, c.D], f32, tag="wgt")
            wq1 = cb.tile([c.D, c.D], f32, tag="wq1")
            wq2 = cb.tile([c.D, 1], f32, tag="wq2")
            bgv = cb.tile([c.D, 1], f32, tag="bgv")
            bgt = cb.tile([c.D, 1], f32, tag="bgt")
            bq1 = cb.tile([c.D, 1], f32, tag="bq1")
            nc.sync.dma_start(wgv[:], gvW[:])
            nc.sync.dma_start(wgt[:], gtW[:])
            nc.sync.dma_start(wq1[:], q1W[:])
            nc.sync.dma_start(wq2[:], q2W[:])
            nc.sync.dma_start(bgv[:], gvB[:])
            nc.sync.dma_start(bgt[:], gtB[:])
            nc.sync.dma_start(bq1[:], q1B[:])

            psq_ctx = tc.tile_pool(name="psq", bufs=1, space="PSUM")
            psq = psq_ctx.__enter__()
            # denom / cnt row vectors
            dT = psq.tile([1, c.BS], f32, tag="q0", name="dT")
            nc.tensor.matmul(dT[:], lhsT=ones50[:], rhs=mkTt[:],
                             start=True, stop=True)
            invd = cb.tile([1, c.BS], f32, tag="invd")
            nc.vector.reciprocal(invd[:], dT[:])
            cT = psq.tile([1, c.BS], f32, tag="q0", name="cT")
            nc.tensor.matmul(cT[:], lhsT=ones50[:], rhs=minTt[:],
                             start=True, stop=True)
            cntR = cb.tile([1, c.BS], b16, tag="cntR")
            nc.vector.tensor_copy(cntR[:], cT[:])

            # ================= Phase A: project U_c rows =================
            psA_ctx = tc.tile_pool(name="psA", bufs=2, space="PSUM")
            psA = psA_ctx.__enter__()
            psB_ctx = tc.tile_pool(name="psB", bufs=2, space="PSUM")
            psB = psB_ctx.__enter__()
            for cc in range(c.ACH):
                v0 = cc * 512
                ai = ap.tile([c.KIW, c.KI * 512], b16, tag="ai")
                for k in range(c.KI):
                    nc.sync.dma_start(ai[:, k * 512:(k + 1) * 512],
                                      imgRT[:, cc, k, :])
                at = ap.tile([c.KTW, c.KT * 512], b16, tag="at")
                for k in range(c.KT):
                    nc.sync.dma_start(at[:, k * 512:(k + 1) * 512],
                                      txtRT[:, cc, k, :])
                ae = ap.tile([c.D, 512], f32, tag="ae")
                nc.sync.dma_start(ae[:], embRT[:, cc, :])
                pst = psA.tile([c.D, 512], f32, tag="psA")
                for k in range(c.KI):
                    nc.tensor.matmul(pst[:], lhsT=wi01t[k][:],
                                     rhs=ai[:, k * 512:(k + 1) * 512],
                                     start=(k == 0), stop=False)
                for k in range(c.KT):
                    nc.tensor.matmul(pst[:], lhsT=wt015t[k][:],
                                     rhs=at[:, k * 512:(k + 1) * 512],
                                     start=False, stop=(k == c.KT - 1))
                io = ost.tile([c.D, 512], f32, tag="io")
                nc.vector.tensor_add(io[:], pst[:], ae[:])
                st = ost.tile([128, 512], b16, tag="st")
                for j in range(4):
                    trj = psB.tile([128, 128], f32, tag="trj")
                    nc.tensor.transpose(trj[:], io[:, j * 128:(j + 1) * 128],
                                        identf[:])
                    if j % 2 == 0:
                        nc.scalar.copy(st[:, j * 128:(j + 1) * 128], trj[:])
                    else:
                        nc.vector.tensor_copy(st[:, j * 128:(j + 1) * 128], trj[:])
                nc.sync.dma_start(
                    localI[v0:v0 + 512, :].rearrange("(n p) d -> p n d", n=4),
                    st[:].rearrange("p (n d) -> p n d", n=4))

            psB_ctx.__exit__(None, None, None)
            psA_ctx.__exit__(None, None, None)

            # ================= AllGather (bf16 item table) =================
            nc.gpsimd.collective_compute(
                "AllGather", mybir.AluOpType.bypass, replica_groups=rg,
                ins=[localI[:].opt()], outs=[Titem[1:c.NF, :].opt()])

            # ========== session raw sums (independent of phase A / AG) =====
            Sraw = cb.tile([c.BS, c.RAWW], f32, tag="Sraw")
            psS_ctx = tc.tile_pool(name="psS", bufs=2, space="PSUM")
            psS = psS_ctx.__enter__()
            GP = 1   # pairs per gather
            for pg_ in range(c.NPAIR // GP):
                gvi = cg.tile([c.L2, GP * c.RAWW], mybir.dt.float8e4,
                              tag="gvi")
                nc.gpsimd.indirect_dma_start(
                    out=gvi[:], out_offset=None, in_=rawcat[:],
                    in_offset=bass.IndirectOffsetOnAxis(
                        ap=six[:, pg_:pg_ + 1], axis=0))
                for pp in range(GP):
                    p = pg_ * GP + pp
                    stg = cs.tile([2, c.RAWW], f32, tag="stg")
                    for jj, (s0, w) in enumerate(SEG):
                        pss = psS.tile([2, 512], f32, tag="pss")
                        nc.tensor.matmul(
                            pss[:, 0:w], lhsT=m16[:, 2 * p:2 * p + 2],
                            rhs=gvi[:, pp * c.RAWW + s0:pp * c.RAWW + s0 + w],
                            start=True, stop=True)
                        if jj % 2 == 0:
                            nc.scalar.copy(stg[:, s0:s0 + w], pss[:, 0:w])
                        else:
                            nc.vector.tensor_copy(stg[:, s0:s0 + w],
                                                  pss[:, 0:w])
                    nc.sync.dma_start(Sraw[2 * p:2 * p + 2, :], stg[:])

            # h0 gathers (after raw gathers in the gpsimd queue; depend on
            # the AllGather)
            h0all = cb.tile([c.L2, c.NPAIR * c.D], b16, tag="h0all")
            for p in range(c.NPAIR):
                nc.gpsimd.indirect_dma_start(
                    out=h0all[:, p * c.D:(p + 1) * c.D], out_offset=None,
                    in_=Titem[:],
                    in_offset=bass.IndirectOffsetOnAxis(
                        ap=hix[:, p:p + 1], axis=0))

            Gsb = cb.tile([c.L2, c.NPAIR * c.L2], b16, tag="Gsb")
            GTsb = cb.tile([c.L2, c.NPAIR * c.L2], b16, tag="GTsb")
            nc.sync.dma_start(
                Gsb[:].rearrange("l (p e) -> l p e", p=c.NPAIR),
                Gbd.rearrange("p l e -> l p e"))
            nc.sync.dma_start(
                GTsb[:].rearrange("l (p e) -> l p e", p=c.NPAIR),
                GTbd.rearrange("p l e -> l p e"))

            # transpose Sraw -> SrawT chunks [d-chunk, 64] bf16
            psT_ctx = tc.tile_pool(name="psT", bufs=2, space="PSUM")
            psT = psT_ctx.__enter__()
            SrawT = cb.tile([128, len(CH) * c.BS], b16, tag="SrawT")
            for ch, (s0, w) in enumerate(CH):
                trS = psT.tile([128, c.BS], f32, tag="trS")
                nc.tensor.transpose(trS[0:w, :], Sraw[:, s0:s0 + w],
                                    identf[0:c.BS, 0:c.BS])
                nc.scalar.copy(SrawT[0:w, ch * c.BS:(ch + 1) * c.BS],
                               trS[0:w, :])
            psT_ctx.__exit__(None, None, None)
            psS_ctx.__exit__(None, None, None)

            # ========== project session sums:  X = W^T S + b*cnt ==========
            psg_ctx = tc.tile_pool(name="psg", bufs=2, space="PSUM")
            psg = psg_ctx.__enter__()
            XimP = psg.tile([c.D, c.BS], f32, tag="px", name="XimP")
            for k in range(c.KI):
                nc.tensor.matmul(XimP[:], lhsT=wiNt[k][:],
                                 rhs=SrawT[0:c.KIW, k * c.BS:(k + 1) * c.BS],
                                 start=(k == 0), stop=False)
            nc.tensor.matmul(XimP[:], lhsT=bir_[:], rhs=cntR[:],
                             start=False, stop=True)
            Xim = cb.tile([c.D, c.BS], f32, tag="Xim")
            nc.scalar.copy(Xim[:], XimP[:])
            XtxP = psg.tile([c.D, c.BS], f32, tag="px", name="XtxP")
            for k in range(c.KT):
                nc.tensor.matmul(
                    XtxP[:], lhsT=wtNt[k][:],
                    rhs=SrawT[0:c.KTW, (c.KI + k) * c.BS:(c.KI + k + 1) * c.BS],
                    start=(k == 0), stop=False)
            nc.tensor.matmul(XtxP[:], lhsT=btr[:], rhs=cntR[:],
                             start=False, stop=True)
            Xtx = cb.tile([c.D, c.BS], f32, tag="Xtx")
            nc.scalar.copy(Xtx[:], XtxP[:])
            # Xit = Semb + 0.1 Xim + 0.15 Xtx
            sembc = cs.tile([c.D, c.BS], f32, tag="sembc")
            nc.scalar.copy(sembc[:],
                           SrawT[:, (c.KI + c.KT) * c.BS:(c.KI + c.KT + 1) * c.BS])
            t1 = cs.tile([c.D, c.BS], f32, tag="t1x")
            nc.vector.tensor_scalar_mul(t1[:], Xim[:], 0.1)
            t2 = cs.tile([c.D, c.BS], f32, tag="t2x")
            nc.vector.tensor_scalar_mul(t2[:], Xtx[:], 0.15)
            Xit = cb.tile([c.D, c.BS], f32, tag="Xit")
            nc.vector.tensor_add(Xit[:], t1[:], t2[:])
            nc.vector.tensor_add(Xit[:], Xit[:], sembc[:])

            # ================= C2: session fusion (as v1) =================
            onesf = cb.tile([1, c.D], f32, tag="onesf")
            nc.vector.memset(onesf[:], 1.0)

            def rep_row(row):
                rp = psg.tile([c.D, c.BS], f32, tag="rep", name="rp")
                nc.tensor.matmul(rp[:], lhsT=onesf[:], rhs=row,
                                 start=True, stop=True)
                return rp

            Xim_m = cb.tile([c.D, c.BS], f32, tag="Xim_m")
            Xtx_m = cb.tile([c.D, c.BS], f32, tag="Xtx_m")
            Xit_m = cb.tile([c.D, c.BS], f32, tag="Xit_m")
            ir = rep_row(invd[:])
            nc.vector.tensor_tensor(Xim_m[:], Xim[:], ir[:], op=OP.mult)
            nc.vector.tensor_tensor(Xtx_m[:], Xtx[:], ir[:], op=OP.mult)
            nc.vector.tensor_tensor(Xit_m[:], Xit[:], ir[:], op=OP.mult)

            pgv = psg.tile([c.D, c.BS], f32, tag="pg")
            nc.tensor.matmul(pgv[:], lhsT=wgv[:], rhs=Xim_m[:],
                             start=True, stop=True)
            gv1 = cs.tile([c.D, c.BS], f32, tag="gv1")
            nc.scalar.activation(gv1[:], pgv[:], AF.Sigmoid,
                                 bias=bgv[:, :1], scale=2.0)
            pgt = psg.tile([c.D, c.BS], f32, tag="pg")
            nc.tensor.matmul(pgt[:], lhsT=wgt[:], rhs=Xtx_m[:],
                             start=True, stop=True)
            gt1 = cs.tile([c.D, c.BS], f32, tag="gt1")
            nc.scalar.activation(gt1[:], pgt[:], AF.Sigmoid,
                                 bias=bgt[:, :1], scale=2.0)
            sid = cb.tile([c.D, c.BS], f32, tag="sid")
            std = cb.tile([c.D, c.BS], f32, tag="std")
            nc.vector.tensor_mul(sid[:], Xit_m[:], gv1[:])
            nc.vector.tensor_mul(std[:], Xit_m[:], gt1[:])

            def qc(xin, tag):
                pq = psg.tile([c.D, c.BS], f32, tag="pg")
                nc.tensor.matmul(pq[:], lhsT=wq1[:], rhs=xin[:],
                                 start=True, stop=True)
                th = cs.tile([c.D, c.BS], f32, tag="th")
                nc.scalar.activation(th[:], pq[:], AF.Tanh,
                                     bias=bq1[:, :1], scale=1.0)
                qq = psq.tile([1, c.BS], f32, tag="q0", name="qq" + tag)
                nc.tensor.matmul(qq[:], lhsT=wq2[:], rhs=th[:],
                                 start=True, stop=True)
                qv = cs.tile([1, c.BS], f32, tag="qv" + tag)
                nc.vector.tensor_copy(qv[:], qq[:])
                return qv

            q1v = qc(sid, "a")
            q2v = qc(std, "b")
            qm = cs.tile([1, c.BS], f32, tag="qm")
            nc.vector.tensor_tensor(qm[:], q1v[:], q2v[:], op=OP.max)
            e1 = cs.tile([1, c.BS], f32, tag="e1")
            e2 = cs.tile([1, c.BS], f32, tag="e2")
            nc.vector.tensor_sub(e1[:], q1v[:], qm[:])
            nc.vector.tensor_sub(e2[:], q2v[:], qm[:])
            nc.scalar.activation(e1[:], e1[:], AF.Exp)
            nc.scalar.activation(e2[:], e2[:], AF.Exp)
            esum = cs.tile([1, c.BS], f32, tag="esum")
            nc.vector.tensor_add(esum[:], e1[:], e2[:])
            rsum = cs.tile([1, c.BS], f32, tag="rsum")
            nc.vector.reciprocal(rsum[:], esum[:])
            w1 = cs.tile([1, c.BS], f32, tag="w1")
            w2 = cs.tile([1, c.BS], f32, tag="w2")
            nc.vector.tensor_mul(w1[:], e1[:], rsum[:])
            nc.vector.tensor_mul(w2[:], e2[:], rsum[:])

            com = cb.tile([c.D, c.BS], f32, tag="com")
            tmp1 = cs.tile([c.D, c.BS], f32, tag="tmp1")
            w1r = rep_row(w1[:])
            nc.vector.tensor_tensor(com[:], sid[:], w1r[:], op=OP.mult)
            w2r = rep_row(w2[:])
            nc.vector.tensor_tensor(tmp1[:], std[:], w2r[:], op=OP.mult)
            nc.vector.tensor_add(com[:], com[:], tmp1[:])

            pg2 = psg.tile([c.D, c.BS], f32, tag="pg")
            nc.tensor.matmul(pg2[:], lhsT=wgv[:], rhs=Xit_m[:],
                             start=True, stop=True)
            gv2 = cs.tile([c.D, c.BS], f32, tag="gv2")
            nc.scalar.activation(gv2[:], pg2[:], AF.Sigmoid,
                                 bias=bgv[:, :1], scale=1.0)
            pg3 = psg.tile([c.D, c.BS], f32, tag="pg")
            nc.tensor.matmul(pg3[:], lhsT=wgt[:], rhs=Xit_m[:],
                             start=True, stop=True)
            gt2 = cs.tile([c.D, c.BS], f32, tag="gt2")
            nc.scalar.activation(gt2[:], pg3[:], AF.Sigmoid,
                                 bias=bgt[:, :1], scale=1.0)

            sep = cs.tile([c.D, c.BS], f32, tag="sep")
            nc.vector.tensor_sub(sep[:], sid[:], com[:])
            nc.vector.tensor_mul(sep[:], gv2[:], sep[:])
            sep2 = cs.tile([c.D, c.BS], f32, tag="sep2")
            nc.vector.tensor_sub(sep2[:], std[:], com[:])
            nc.vector.tensor_mul(sep2[:], gt2[:], sep2[:])
            fus = cs.tile([c.D, c.BS], f32, tag="fus")
            nc.vector.tensor_add(fus[:], sep[:], sep2[:])
            nc.vector.tensor_add(fus[:], fus[:], com[:])
            nc.vector.tensor_scalar_mul(fus[:], fus[:], 1.0 / 3.0)
            Xs = cb.tile([c.D, c.BS], f32, tag="Xs")
            nc.vector.tensor_add(Xs[:], Xit_m[:], Xim_m[:])
            nc.vector.tensor_add(Xs[:], Xs[:], Xtx_m[:])
            nc.vector.tensor_add(Xs[:], Xs[:], fus[:])

            # degrees (bf16 G blocks, f32 out)
            idnA = cb.tile([c.L2, c.NPAIR], f32, tag="idnA")
            ideA = cb.tile([c.L2, c.NPAIR], f32, tag="ideA")
            dtmp = cs.tile([c.L2, 2], f32, tag="dtmp")
            for p in range(c.NPAIR):
                nc.vector.reduce_sum(dtmp[:, 0:1],
                                     Gsb[:, p * c.L2:(p + 1) * c.L2], axis=AX.X)
                nc.vector.reduce_sum(dtmp[:, 1:2],
                                     GTsb[:, p * c.L2:(p + 1) * c.L2], axis=AX.X)
                nc.vector.reciprocal(idnA[:, p:p + 1], dtmp[:, 0:1])
                nc.vector.reciprocal(ideA[:, p:p + 1], dtmp[:, 1:2])

            # Xs -> per-pair session rows [2, 32*128] via one transpose + DMA
            trx = psg.tile([c.BS, c.D], f32, tag="px", name="trx")
            nc.tensor.transpose(trx[:], Xs[:], identf[:])
            XsSb = cs.tile([c.BS, c.D], b16, tag="XsSb")
            nc.vector.tensor_copy(XsSb[:], trx[:])
            nc.sync.dma_start(XsDram[:], XsSb[:])
            Xrows = cb.tile([2, c.NPAIR * c.D], b16, tag="Xrows")
            nc.sync.dma_start(
                Xrows[:].rearrange("j (p d) -> j p d", p=c.NPAIR),
                XsDram.rearrange("(p j) d -> j p d", p=c.NPAIR))

            psg_ctx.__exit__(None, None, None)
            psq_ctx.__exit__(None, None, None)

            # ================= C3: hypergraph layers =================
            with (
                tc.tile_pool(name="psR", bufs=3, space="PSUM") as psR,
                tc.tile_pool(name="psE", bufs=5, space="PSUM") as psE,
            ):
                for p in range(c.NPAIR):
                    Gp = Gsb[:, p * c.L2:(p + 1) * c.L2]
                    GTp = GTsb[:, p * c.L2:(p + 1) * c.L2]
                    srep = psR.tile([c.L2, c.D], f32, tag="srep")
                    nc.tensor.matmul(srep[:], lhsT=i2[:],
                                     rhs=Xrows[:, p * c.D:(p + 1) * c.D],
                                     start=True, stop=True)
                    hcur = h0all[:, p * c.D:(p + 1) * c.D]
                    for lyr in range(2):
                        pe_ = psE.tile([c.L2, c.D], f32, tag="pe")
                        nc.tensor.matmul(pe_[:], lhsT=Gp, rhs=hcur,
                                         start=True, stop=True)
                        ee = cs.tile([c.L2, c.D], b16, tag="ee")
                        nc.scalar.activation(ee[:], pe_[:], AF.Copy,
                                             scale=ideA[:, p:p + 1])
                        ph_ = psE.tile([c.L2, c.D], f32, tag="pe",
                                       name="ph")
                        nc.tensor.matmul(ph_[:], lhsT=GTp, rhs=ee[:],
                                         start=True, stop=True)
                        hs = cs.tile([c.L2, c.D], f32, tag=f"hs{lyr}")
                        nc.vector.tensor_scalar_mul(hs[:], ph_[:],
                                                    idnA[:, p:p + 1])
                        if lyr == 0:
                            hh = cs.tile([c.L2, c.D], b16, tag="hh0")
                            nc.vector.tensor_add(hh[:], hs[:], srep[:])
                            hcur = hh[:]
                        else:
                            hh2 = cs.tile([c.L2, c.D], f32, tag="hh1")
                            nc.vector.tensor_add(hh2[:], hs[:], srep[:])
                            nc.sync.dma_start(
                                outH[2 * p:2 * p + 2].rearrange(
                                    "b l d -> (b l) d"), hh2[:])
    nc.compile()
    return nc


_CACHE = {}


def _get_program(c: Cfg):
    key = (c.N, c.B)
    if key not in _CACHE:
        _CACHE[key] = build_program(c)
    return _CACHE[key]


def _prep_inputs(c: Cfg, inputs, item, mask_item, Hs, emb_table, img_table,
                 txt_table, img_W, img_b, txt_W, txt_b, gate_v_W, gate_v_b,
                 gate_t_W, gate_t_b, qc_W1, qc_b1, qc_W2):
    f32 = np.float32
    inputs = np.asarray(inputs); item = np.asarray(item)
    # U = referenced vocab ids (>=1 in padded space; id 0 is the pad row)
    U = np.unique(np.concatenate([inputs.ravel(), item.ravel()]))
    U = U[U > 0].astype(np.int64)
    NU = len(U)
    assert NU <= c.NU, f"unique referenced rows {NU} > capacity {c.NU}"
    Upad = np.concatenate([U, np.full(c.NU - NU, U[-1], np.int64)])
    pos = np.zeros(c.N + 1, np.int32)
    pos[U] = np.arange(1, NU + 1, dtype=np.int32)
    h0x = pos[inputs]                       # [B, L] U-space indices
    ssx = pos[item]

    rows = Upad - 1
    FP8 = ml_dtypes.float8_e4m3fn
    rawcat = np.zeros((c.NF, c.RAWW), dtype=FP8)
    rawcat[1:, :c.IMG] = img_table[rows].astype(FP8)
    rawcat[1:, c.IMG:c.IMG + c.TXT] = txt_table[rows].astype(FP8)
    rawcat[1:, c.IMG + c.TXT:] = emb_table[rows].astype(FP8)

    maskf = np.asarray(mask_item).astype(f32)
    bcomb = (0.1 * img_b + 0.15 * txt_b).astype(f32).reshape(c.D, 1)
    # bias baked into the transposed emb rows (applies to every real row)
    in_maps = []
    for kk in range(c.NC):
        Uc = Upad[kk * c.UC:(kk + 1) * c.UC] - 1
        b0, b1 = kk * c.BS, (kk + 1) * c.BS
        Hk = np.asarray(Hs[b0:b1]).astype(f32)
        mk = maskf[b0:b1]
        Gbd = np.zeros((c.NPAIR, c.L2, c.L2), f32)
        GTbd = np.zeros((c.NPAIR, c.L2, c.L2), f32)
        Mbd = np.zeros((c.NPAIR, c.L2, 2), f32)
        for p in range(c.NPAIR):
            Gbd[p, :c.L, :c.L] = Hk[2 * p]
            Gbd[p, c.L:, c.L:] = Hk[2 * p + 1]
            GTbd[p, :c.L, :c.L] = Hk[2 * p].T
            GTbd[p, c.L:, c.L:] = Hk[2 * p + 1].T
            Mbd[p, :c.L, 0] = mk[2 * p]
            Mbd[p, c.L:, 1] = mk[2 * p + 1]
        ind2 = np.zeros((2, c.L2), f32)
        ind2[0, :c.L] = 1.0
        ind2[1, c.L:] = 1.0
        mind = mk * (item[b0:b1] > 0)
        in_maps.append({
            "imgRT": np.ascontiguousarray(
                img_table[Uc].astype(BF16).reshape(c.ACH, 512, c.KI, c.KIW)
                .transpose(3, 0, 2, 1)),
            "txtRT": np.ascontiguousarray(
                txt_table[Uc].astype(BF16).reshape(c.ACH, 512, c.KT, c.KTW)
                .transpose(3, 0, 2, 1)),
            "embRT": np.ascontiguousarray(
                (emb_table[Uc].T + bcomb).astype(f32)
                .reshape(c.D, c.ACH, 512)),
            "wiN": img_W.astype(BF16), "wtN": txt_W.astype(BF16),
            "wi01": (0.1 * img_W).astype(BF16),
            "wt015": (0.15 * txt_W).astype(BF16),
            "imgbR": img_b.reshape(1, c.D).astype(BF16),
            "txtbR": txt_b.reshape(1, c.D).astype(BF16),
            "rawcat": rawcat,
            "gvW": gate_v_W.astype(f32), "gvB": gate_v_b.reshape(c.D, 1).astype(f32),
            "gtW": gate_t_W.astype(f32), "gtB": gate_t_b.reshape(c.D, 1).astype(f32),
            "q1W": qc_W1.astype(f32), "q1B": qc_b1.reshape(c.D, 1).astype(f32),
            "q2W": qc_W2.astype(f32),
            "Gbd": Gbd.astype(BF16), "GTbd": GTbd.astype(BF16),
            "Mbd16": Mbd.astype(ml_dtypes.float8_e4m3fn),
            "mkT": np.ascontiguousarray(mk.T).astype(BF16),
            "mindT": np.ascontiguousarray(mind.T).astype(BF16),
            "ind2": ind2.astype(BF16),
            "h0idx": h0x[b0:b1].reshape(c.NPAIR, c.L2, 1),
            "ssidx": ssx[b0:b1].reshape(c.NPAIR, c.L2, 1),
        })
    return in_maps


def run(c: Cfg, trace=False, **inputs):
    nc = _get_program(c)
    in_maps = _prep_inputs(c, **{k: np.asarray(v) for k, v in inputs.items()})
    res = bass_utils.run_bass_kernel_spmd(
        nc, in_maps, core_ids=list(range(c.NC)), trace=trace)
    out = np.concatenate([r["outH"] for r in res.results], axis=0)
    return out.astype(np.float32), res


def kernel(**inputs):
    out, _ = run(REAL, trace=False, **inputs)
    return out



# revision 6
# speedup vs baseline: 2.5391x; 2.5391x over previous
"""Trainium2 Bass kernel for the gnn_message_passing problem, v3.

Strategy (8 NeuronCores, SPMD, fully local — no collectives, no indirect DMA):
  - Each core handles 64 sessions.  The host stages, per core, the raw
    img/txt/emb rows referenced by its own `inputs` (3200 entries, fp8,
    pre-transposed) and `item` (3200 entries, fp8, row-major) in exactly
    the SBUF layout the kernel wants, so every DMA is one contiguous
    descriptor per partition (>=2KB) and the whole kernel is ~20 large
    DMAs split across both HWDGE queues (sync + scalar) plus gpsimd SWDGE
    for small constants.
  - h0 path: project the 3200 input rows with weights-stationary fp8
    matmuls (Wcat = [0.1*img_W; 0.15*txt_W]) + bf16 emb rows (bias baked),
    PE-transpose to pair-major row layout.
  - Session path: masked per-session sums of raw item rows via
    rr-stationary fp8 matmuls that directly produce the TRANSPOSED sums
    ST [col-chunk, 64]; project with bf16 weight chunks (biases as extra
    weight rows against an indicator column).  Fusion math in [128, 64]
    transposed layout.
  - Hypergraph: 2 layers per session pair (block-diagonal [100,100] bf16
    G / G^T), session context injected via selection-matrix matmuls.
"""

import sys

sys.path.insert(0, "/opt/trn_rl_repo")

import numpy as np
import ml_dtypes

import concourse.bass as bass
import concourse.bacc as bacc
import concourse.mybir as mybir
import concourse.tile as tile
from concourse import bass_utils

BF16 = ml_dtypes.bfloat16
FP8 = ml_dtypes.float8_e4m3fn


class Cfg:
    def __init__(self):
        self.N = 50000
        self.D = 128
        self.IMG = 1000
        self.TXT = 768
        self.B = 512
        self.L = 50
        self.NC = 8
        self.BS = self.B // self.NC      # 64 sessions per core
        self.NPAIR = self.BS // 2        # 32
        self.L2 = 2 * self.L             # 100
        self.NE = self.BS * self.L       # 3200 entries per core
        self.KP = 14                     # proj k-chunks (1792 = 14*128)
        self.KR = 25                     # rr row chunks (3200 = 25*128)
        self.RW = 1920                   # rr width (15*128): emb|img|txt|ind|pad
        self.MR = 15                     # rr col chunks
        self.NW = 17                     # session weight chunks


REAL = Cfg()


def build_program(c: Cfg):
    f32 = mybir.dt.float32
    b16 = mybir.dt.bfloat16
    f8 = mybir.dt.float8e4
    AF = mybir.ActivationFunctionType
    AX = mybir.AxisListType
    OP = mybir.AluOpType

    nc = bacc.Bacc("TRN2", target_bir_lowering=False, debug=False,
                   num_devices=c.NC)

    def ein(nm, sh, dt):
        return nc.dram_tensor(nm, sh, dt, kind="ExternalInput")

    prTd = ein("prTd", [128, c.KP * c.NE], f8)      # [k-part, chunk*entry]
    embTd = ein("embTd", [128, c.NE], b16)          # emb'(+bcomb) transposed
    rrd = ein("rrd", [128, c.KR * c.RW], f8)        # item raw rows, row-major chunks
    mkTd = ein("mkTd", [128, c.KR * c.BS], f8)      # per-chunk session mask
    Wcatd = ein("Wcatd", [128, c.KP * c.D], f8)     # [0.1Wi;0.15Wt] chunks
    Wsd = ein("Wsd", [128, c.NW * c.D], b16)        # session proj weight chunks
    Gd = ein("Gd", [c.L2, c.NPAIR * c.L2], b16)
    GTd = ein("GTd", [c.L2, c.NPAIR * c.L2], b16)
    seld = ein("seld", [c.BS, c.NPAIR * c.L2], b16)
    mbld = ein("mbld", [c.L, c.BS], b16)            # mask.T for denom
    gvW = ein("gvW", [c.D, c.D], f32)
    gvB = ein("gvB", [c.D, 1], f32)
    gtW = ein("gtW", [c.D, c.D], f32)
    gtB = ein("gtB", [c.D, 1], f32)
    q1W = ein("q1W", [c.D, c.D], f32)
    q1B = ein("q1B", [c.D, 1], f32)
    q2W = ein("q2W", [c.D, 1], f32)

    outd = nc.dram_tensor("outd", [c.L2, c.NPAIR * c.D], f32,
                          kind="ExternalOutput")

    # proj row chunks (free dim <= 512 per PSUM bank)
    RCH = [(i * 512, 512) for i in range(6)] + [(3072, 128)]
    # session chains: (chunk list in ws, ST chunk list)
    IMCH = list(range(1, 9))     # img rows live in ST chunks 1..8
    TXCH = list(range(8, 15))    # txt rows live in ST chunks 8..14

    with tile.TileContext(nc) as tc:
        with (
            tc.tile_pool(name="wpool", bufs=1) as wp,
            tc.tile_pool(name="cpool", bufs=1) as cb,
            tc.tile_pool(name="cs", bufs=4) as cs,
            tc.tile_pool(name="psA", bufs=2, space="PSUM") as psA,
            tc.tile_pool(name="psS", bufs=2, space="PSUM") as psS,
            tc.tile_pool(name="psB", bufs=2, space="PSUM") as psB,
            tc.tile_pool(name="psE", bufs=2, space="PSUM") as psE,
        ):
            # ---------------- constants / identities ----------------
            identf = wp.tile([128, 128], f32, tag="idf")
            from concourse.masks import make_identity
            make_identity(nc, identf[:])
            ones50 = wp.tile([c.L, 1], b16, tag="ones50")
            nc.vector.memset(ones50[:], 1.0)
            onesf = wp.tile([1, c.D], f32, tag="onesf")
            nc.vector.memset(onesf[:], 1.0)

            # ---------------- big DMAs: sync queue ----------------
            wc = wp.tile([128, c.KP * c.D], f8, tag="wc")
            nc.sync.dma_start(wc[:], Wcatd[:])
            prT = wp.tile([128, c.KP * c.NE], f8, tag="prT")
            QP = [(0, 4), (4, 8), (8, 11), (11, 14)]
            for a, b in QP:
                nc.sync.dma_start(prT[:, a * c.NE:b * c.NE],
                                  prTd[:, a * c.NE:b * c.NE])
            embT = wp.tile([128, c.NE], b16, tag="embT")
            nc.sync.dma_start(embT[:], embTd[:])
            Gsb = wp.tile([c.L2, c.NPAIR * c.L2], b16, tag="Gsb")
            nc.sync.dma_start(Gsb[:], Gd[:])
            GTsb = wp.tile([c.L2, c.NPAIR * c.L2], b16, tag="GTsb")
            nc.sync.dma_start(GTsb[:], GTd[:])

            # ---------------- big DMAs: scalar queue ----------------
            rr = wp.tile([128, c.KR * c.RW], f8, tag="rr")
            QR = [(0, 7), (7, 13), (13, 19), (19, 25)]
            for a, b in QR:
                nc.scalar.dma_start(rr[:, a * c.RW:b * c.RW],
                                    rrd[:, a * c.RW:b * c.RW])

            # ---------------- small DMAs: gpsimd SWDGE ----------------
            mbl = wp.tile([c.L, c.BS], b16, tag="mbl")
            nc.gpsimd.dma_start(mbl[:], mbld[:])
            mkT = wp.tile([128, c.KR * c.BS], f8, tag="mkT")
            nc.gpsimd.dma_start(mkT[:], mkTd[:])
            ws = wp.tile([128, c.NW * c.D], b16, tag="ws")
            nc.gpsimd.dma_start(ws[:], Wsd[:])
            sel = wp.tile([c.BS, c.NPAIR * c.L2], b16, tag="sel")
            nc.gpsimd.dma_start(sel[:], seld[:])
            wgv = cb.tile([c.D, c.D], f32, tag="wgv")
            wgt = cb.tile([c.D, c.D], f32, tag="wgt")
            wq1 = cb.tile([c.D, c.D], f32, tag="wq1")
            wq2 = cb.tile([c.D, 1], f32, tag="wq2")
            bgv = cb.tile([c.D, 1], f32, tag="bgv")
            bgt = cb.tile([c.D, 1], f32, tag="bgt")
            bq1 = cb.tile([c.D, 1], f32, tag="bq1")
            nc.gpsimd.dma_start(wgv[:], gvW[:])
            nc.gpsimd.dma_start(wgt[:], gtW[:])
            nc.gpsimd.dma_start(wq1[:], q1W[:])
            nc.gpsimd.dma_start(wq2[:], q2W[:])
            nc.gpsimd.dma_start(bgv[:], gvB[:])
            nc.gpsimd.dma_start(bgt[:], gtB[:])
            nc.gpsimd.dma_start(bq1[:], q1B[:])

            # ---------------- h0 projection ----------------
            hTs = cb.tile([128, c.NE], f32, tag="hTs")
            for rc, (r0, w) in enumerate(RCH):
                pa = psA.tile([128, 512], f32, tag="psA", name=f"pa{rc}")
                for k in range(c.KP):
                    nc.tensor.matmul(
                        pa[:, 0:w], lhsT=wc[:, k * c.D:(k + 1) * c.D],
                        rhs=prT[:, k * c.NE + r0:k * c.NE + r0 + w],
                        start=(k == 0), stop=(k == c.KP - 1))
                nc.vector.tensor_add(hTs[:, r0:r0 + w], pa[:, 0:w],
                                     embT[:, r0:r0 + w])

            # transpose to pair-major rows: h0 [100, p*128]
            h0 = cb.tile([c.L2, c.NPAIR * c.D], b16, tag="h0")
            for p in range(c.NPAIR):
                tr = psB.tile([128, 512], f32, tag="psB", name=f"tr{p}")
                nc.tensor.transpose(tr[0:c.L2, 0:c.D],
                                    hTs[:, p * c.L2:(p + 1) * c.L2],
                                    identf[:])
                if p % 2 == 0:
                    nc.scalar.copy(h0[:, p * c.D:(p + 1) * c.D],
                                   tr[0:c.L2, 0:c.D])
                else:
                    nc.vector.tensor_copy(h0[:, p * c.D:(p + 1) * c.D],
                                          tr[0:c.L2, 0:c.D])

            # ---------------- session sums (transposed) ----------------
            ST = cb.tile([128, c.MR * c.BS], b16, tag="ST")
            Semb = cb.tile([128, c.BS], f32, tag="Semb")
            for m in range(c.MR):
                ps = psS.tile([128, 512], f32, tag="psS", name=f"ps{m}")
                for k in range(c.KR):
                    nc.tensor.matmul(
                        ps[:, 0:c.BS],
                        lhsT=rr[:, k * c.RW + m * c.D:k * c.RW + (m + 1) * c.D],
                        rhs=mkT[:, k * c.BS:(k + 1) * c.BS],
                        start=(k == 0), stop=(k == c.KR - 1))
                if m == 0:
                    nc.vector.tensor_copy(Semb[:], ps[:, 0:c.BS])
                elif m % 2 == 0:
                    nc.vector.tensor_copy(ST[:, m * c.BS:(m + 1) * c.BS],
                                          ps[:, 0:c.BS])
                else:
                    nc.scalar.copy(ST[:, m * c.BS:(m + 1) * c.BS],
                                   ps[:, 0:c.BS])

            # ---------------- denom ----------------
            dT = psS.tile([128, 512], f32, tag="psS", name="dT")
            nc.tensor.matmul(dT[0:1, 0:c.BS], lhsT=ones50[:], rhs=mbl[:],
                             start=True, stop=True)
            invd = cb.tile([1, c.BS], f32, tag="invd")
            nc.vector.reciprocal(invd[:], dT[0:1, 0:c.BS])

            # ---------------- degree reciprocals ----------------
            idnA = cb.tile([c.L2, c.NPAIR], f32, tag="idnA")
            ideA = cb.tile([c.L2, c.NPAIR], f32, tag="ideA")
            dtmp = cs.tile([c.L2, 2], f32, tag="dtmp")
            for p in range(c.NPAIR):
                nc.vector.reduce_sum(dtmp[:, 0:1],
                                     Gsb[:, p * c.L2:(p + 1) * c.L2], axis=AX.X)
                nc.vector.reduce_sum(dtmp[:, 1:2],
                                     GTsb[:, p * c.L2:(p + 1) * c.L2], axis=AX.X)
                nc.vector.reciprocal(idnA[:, p:p + 1], dtmp[:, 0:1])
                nc.vector.reciprocal(ideA[:, p:p + 1], dtmp[:, 1:2])

            # ---------------- session projections ----------------
            # Xim' = sum_img_chunks Wi.T @ ST ; Xim = Xim' + bi (x) cnt
            pim = psA.tile([128, 512], f32, tag="psA", name="pim")
            for i, m in enumerate(IMCH):
                nc.tensor.matmul(pim[:, 0:c.BS],
                                 lhsT=ws[:, i * c.D:(i + 1) * c.D],
                                 rhs=ST[:, m * c.BS:(m + 1) * c.BS],
                                 start=(i == 0), stop=False)
            XimQ = cs.tile([c.D, c.BS], f32, tag="XimQ")
            nc.scalar.copy(XimQ[:], pim[:, 0:c.BS])
            nc.tensor.matmul(pim[:, 0:c.BS], lhsT=ws[:, 8 * c.D:9 * c.D],
                             rhs=ST[:, 14 * c.BS:15 * c.BS],
                             start=False, stop=True)
            Xim = cb.tile([c.D, c.BS], f32, tag="Xim")
            nc.vector.tensor_copy(Xim[:], pim[:, 0:c.BS])

            ptx = psA.tile([128, 512], f32, tag="psA", name="ptx")
            for i, m in enumerate(TXCH):
                nc.tensor.matmul(ptx[:, 0:c.BS],
                                 lhsT=ws[:, (9 + i) * c.D:(10 + i) * c.D],
                                 rhs=ST[:, m * c.BS:(m + 1) * c.BS],
                                 start=(i == 0), stop=False)
            XtxQ = cs.tile([c.D, c.BS], f32, tag="XtxQ")
            nc.scalar.copy(XtxQ[:], ptx[:, 0:c.BS])
            nc.tensor.matmul(ptx[:, 0:c.BS], lhsT=ws[:, 16 * c.D:17 * c.D],
                             rhs=ST[:, 14 * c.BS:15 * c.BS],
                             start=False, stop=True)
            Xtx = cb.tile([c.D, c.BS], f32, tag="Xtx")
            nc.vector.tensor_copy(Xtx[:], ptx[:, 0:c.BS])

            # Xit = Semb' + 0.1 Xim' + 0.15 Xtx'
            t1 = cs.tile([c.D, c.BS], f32, tag="t1x")
            nc.vector.tensor_scalar_mul(t1[:], XimQ[:], 0.1)
            t2 = cs.tile([c.D, c.BS], f32, tag="t2x")
            nc.vector.tensor_scalar_mul(t2[:], XtxQ[:], 0.15)
            Xit = cb.tile([c.D, c.BS], f32, tag="Xit")
            nc.vector.tensor_add(Xit[:], t1[:], t2[:])
            nc.vector.tensor_add(Xit[:], Xit[:], Semb[:])

            # ---------------- fusion (transposed [128, 64]) ----------------
            def rep_row(row, nm):
                rp = psA.tile([128, 512], f32, tag="psA", name=nm)
                nc.tensor.matmul(rp[:, 0:c.BS], lhsT=onesf[:], rhs=row,
                                 start=True, stop=True)
                return rp

            Xim_m = cb.tile([c.D, c.BS], f32, tag="Xim_m")
            Xtx_m = cb.tile([c.D, c.BS], f32, tag="Xtx_m")
            Xit_m = cb.tile([c.D, c.BS], f32, tag="Xit_m")
            ir = rep_row(invd[:], "ir")
            nc.vector.tensor_tensor(Xim_m[:], Xim[:], ir[:, 0:c.BS], op=OP.mult)
            nc.vector.tensor_tensor(Xtx_m[:], Xtx[:], ir[:, 0:c.BS], op=OP.mult)
            nc.vector.tensor_tensor(Xit_m[:], Xit[:], ir[:, 0:c.BS], op=OP.mult)

            pgv = psA.tile([128, 512], f32, tag="psA", name="pgv")
            nc.tensor.matmul(pgv[:, 0:c.BS], lhsT=wgv[:], rhs=Xim_m[:],
                             start=True, stop=True)
            gv1 = cs.tile([c.D, c.BS], f32, tag="gv1")
            nc.scalar.activation(gv1[:], pgv[:, 0:c.BS], AF.Sigmoid,
                                 bias=bgv[:, :1], scale=2.0)
            pgt = psA.tile([128, 512], f32, tag="psA", name="pgt")
            nc.tensor.matmul(pgt[:, 0:c.BS], lhsT=wgt[:], rhs=Xtx_m[:],
                             start=True, stop=True)
            gt1 = cs.tile([c.D, c.BS], f32, tag="gt1")
            nc.scalar.activation(gt1[:], pgt[:, 0:c.BS], AF.Sigmoid,
                                 bias=bgt[:, :1], scale=2.0)
            sid = cb.tile([c.D, c.BS], f32, tag="sid")
            std = cb.tile([c.D, c.BS], f32, tag="std")
            nc.vector.tensor_mul(sid[:], Xit_m[:], gv1[:])
            nc.vector.tensor_mul(std[:], Xit_m[:], gt1[:])

            def qc(xin, tag):
                pq = psA.tile([128, 512], f32, tag="psA", name="pq" + tag)
                nc.tensor.matmul(pq[:, 0:c.BS], lhsT=wq1[:], rhs=xin[:],
                                 start=True, stop=True)
                th = cs.tile([c.D, c.BS], f32, tag="th")
                nc.scalar.activation(th[:], pq[:, 0:c.BS], AF.Tanh,
                                     bias=bq1[:, :1], scale=1.0)
                qq = psS.tile([128, 512], f32, tag="psS", name="qq" + tag)
                nc.tensor.matmul(qq[0:1, 0:c.BS], lhsT=wq2[:], rhs=th[:],
                                 start=True, stop=True)
                qv = cs.tile([1, c.BS], f32, tag="qv" + tag)
                nc.vector.tensor_copy(qv[:], qq[0:1, 0:c.BS])
                return qv

            q1v = qc(sid, "a")
            q2v = qc(std, "b")
            qm = cs.tile([1, c.BS], f32, tag="qm")
            nc.vector.tensor_tensor(qm[:], q1v[:], q2v[:], op=OP.max)
            e1 = cs.tile([1, c.BS], f32, tag="e1")
            e2 = cs.tile([1, c.BS], f32, tag="e2")
            nc.vector.tensor_sub(e1[:], q1v[:], qm[:])
            nc.vector.tensor_sub(e2[:], q2v[:], qm[:])
            nc.scalar.activation(e1[:], e1[:], AF.Exp)
            nc.scalar.activation(e2[:], e2[:], AF.Exp)
            esum = cs.tile([1, c.BS], f32, tag="esum")
            nc.vector.tensor_add(esum[:], e1[:], e2[:])
            rsum = cs.tile([1, c.BS], f32, tag="rsum")
            nc.vector.reciprocal(rsum[:], esum[:])
            w1 = cs.tile([1, c.BS], f32, tag="w1")
            w2 = cs.tile([1, c.BS], f32, tag="w2")
            nc.vector.tensor_mul(w1[:], e1[:], rsum[:])
            nc.vector.tensor_mul(w2[:], e2[:], rsum[:])

            com = cb.tile([c.D, c.BS], f32, tag="com")
            tmp1 = cs.tile([c.D, c.BS], f32, tag="tmp1")
            w1r = rep_row(w1[:], "w1r")
            nc.vector.tensor_tensor(com[:], sid[:], w1r[:, 0:c.BS], op=OP.mult)
            w2r = rep_row(w2[:], "w2r")
            nc.vector.tensor_tensor(tmp1[:], std[:], w2r[:, 0:c.BS], op=OP.mult)
            nc.vector.tensor_add(com[:], com[:], tmp1[:])

            pg2 = psA.tile([128, 512], f32, tag="psA", name="pg2")
            nc.tensor.matmul(pg2[:, 0:c.BS], lhsT=wgv[:], rhs=Xit_m[:],
                             start=True, stop=True)
            gv2 = cs.tile([c.D, c.BS], f32, tag="gv2")
            nc.scalar.activation(gv2[:], pg2[:, 0:c.BS], AF.Sigmoid,
                                 bias=bgv[:, :1], scale=1.0)
            pg3 = psA.tile([128, 512], f32, tag="psA", name="pg3")
            nc.tensor.matmul(pg3[:, 0:c.BS], lhsT=wgt[:], rhs=Xit_m[:],
                             start=True, stop=True)
            gt2 = cs.tile([c.D, c.BS], f32, tag="gt2")
            nc.scalar.activation(gt2[:], pg3[:, 0:c.BS], AF.Sigmoid,
                                 bias=bgt[:, :1], scale=1.0)

            sep = cs.tile([c.D, c.BS], f32, tag="sep")
            nc.vector.tensor_sub(sep[:], sid[:], com[:])
            nc.vector.tensor_mul(sep[:], gv2[:], sep[:])
            sep2 = cs.tile([c.D, c.BS], f32, tag="sep2")
            nc.vector.tensor_sub(sep2[:], std[:], com[:])
            nc.vector.tensor_mul(sep2[:], gt2[:], sep2[:])
            fus = cs.tile([c.D, c.BS], f32, tag="fus")
            nc.vector.tensor_add(fus[:], sep[:], sep2[:])
            nc.vector.tensor_add(fus[:], fus[:], com[:])
            nc.vector.tensor_scalar_mul(fus[:], fus[:], 1.0 / 3.0)
            Xs = cb.tile([c.D, c.BS], f32, tag="Xs")
            nc.vector.tensor_add(Xs[:], Xit_m[:], Xim_m[:])
            nc.vector.tensor_add(Xs[:], Xs[:], Xtx_m[:])
            nc.vector.tensor_add(Xs[:], Xs[:], fus[:])

            # transpose Xs -> XsT [64, 128] bf16
            trx = psB.tile([128, 512], f32, tag="psB", name="trx")
            nc.tensor.transpose(trx[0:c.BS, 0:c.D], Xs[:], identf[:])
            XsT = cb.tile([c.BS, c.D], b16, tag="XsT")
            nc.vector.tensor_copy(XsT[:], trx[0:c.BS, 0:c.D])

            # ---------------- hypergraph layer-1 e-steps ----------------
            eeS = cb.tile([c.L2, c.NPAIR * c.D], b16, tag="eeS")
            for p in range(c.NPAIR):
                pe1 = psE.tile([128, 512], f32, tag="psE", name=f"pe1{p}")
                nc.tensor.matmul(pe1[0:c.L2, 0:c.D],
                                 lhsT=Gsb[:, p * c.L2:(p + 1) * c.L2],
                                 rhs=h0[:, p * c.D:(p + 1) * c.D],
                                 start=True, stop=True)
                nc.scalar.activation(eeS[:, p * c.D:(p + 1) * c.D],
                                     pe1[0:c.L2, 0:c.D], AF.Copy,
                                     scale=ideA[:, p:p + 1])

            # ---------------- hypergraph main loop ----------------
            houtf = cb.tile([c.L2, c.NPAIR * c.D], f32, tag="houtf")
            for p in range(c.NPAIR):
                srep = psB.tile([128, 512], f32, tag="psB", name=f"sr{p}")
                nc.tensor.matmul(srep[0:c.L2, 0:c.D],
                                 lhsT=sel[:, p * c.L2:(p + 1) * c.L2],
                                 rhs=XsT[:], start=True, stop=True)
                ph1 = psE.tile([128, 512], f32, tag="psE", name=f"ph1{p}")
                nc.tensor.matmul(ph1[0:c.L2, 0:c.D],
                                 lhsT=GTsb[:, p * c.L2:(p + 1) * c.L2],
                                 rhs=eeS[:, p * c.D:(p + 1) * c.D],
                                 start=True, stop=True)
                tmf = cs.tile([c.L2, c.D], f32, tag="tmf")
                nc.vector.tensor_scalar_mul(tmf[:], ph1[0:c.L2, 0:c.D],
                                            idnA[:, p:p + 1])
                hh1 = cs.tile([c.L2, c.D], b16, tag="hh1")
                nc.vector.tensor_add(hh1[:], tmf[:], srep[0:c.L2, 0:c.D])
                pe2 = psE.tile([128, 512], f32, tag="psE", name=f"pe2{p}")
                nc.tensor.matmul(pe2[0:c.L2, 0:c.D],
                                 lhsT=Gsb[:, p * c.L2:(p + 1) * c.L2],
                                 rhs=hh1[:], start=True, stop=True)
                ee2 = cs.tile([c.L2, c.D], b16, tag="ee2")
                nc.scalar.activation(ee2[:], pe2[0:c.L2, 0:c.D], AF.Copy,
                                     scale=ideA[:, p:p + 1])
                ph2 = psE.tile([128, 512], f32, tag="psE", name=f"ph2{p}")
                nc.tensor.matmul(ph2[0:c.L2, 0:c.D],
                                 lhsT=GTsb[:, p * c.L2:(p + 1) * c.L2],
                                 rhs=ee2[:], start=True, stop=True)
                tmf2 = cs.tile([c.L2, c.D], f32, tag="tmf2")
                nc.vector.tensor_scalar_mul(tmf2[:], ph2[0:c.L2, 0:c.D],
                                            idnA[:, p:p + 1])
                nc.vector.tensor_add(houtf[:, p * c.D:(p + 1) * c.D],
                                     tmf2[:], srep[0:c.L2, 0:c.D])
                if p % 8 == 7:
                    g0 = (p - 7) * c.D
                    g1 = (p + 1) * c.D
                    nc.scalar.dma_start(outd[:, g0:g1], houtf[:, g0:g1])
    nc.compile()
    return nc


_CACHE = {}


def _get_program(c: Cfg):
    key = (c.N, c.B)
    if key not in _CACHE:
        _CACHE[key] = build_program(c)
    return _CACHE[key]


def _prep_inputs(c: Cfg, inputs, item, mask_item, Hs, emb_table, img_table,
                 txt_table, img_W, img_b, txt_W, txt_b, gate_v_W, gate_v_b,
                 gate_t_W, gate_t_b, qc_W1, qc_b1, qc_W2):
    f32 = np.float32
    inputs = np.asarray(inputs)
    item = np.asarray(item)
    maskf = np.asarray(mask_item).astype(f32)
    Hs = np.asarray(Hs).astype(f32)
    emb_table = np.asarray(emb_table).astype(f32)
    img_table = np.asarray(img_table).astype(f32)
    txt_table = np.asarray(txt_table).astype(f32)
    bcomb = (0.1 * np.asarray(img_b) + 0.15 * np.asarray(txt_b)).astype(f32)

    # Wcat: [0.1*img_W ; 0.15*txt_W ; zero-pad to 1792] -> [128, 14, 128] fp8
    Wc = np.zeros((c.KP * 128, c.D), f32)
    Wc[:c.IMG] = 0.1 * np.asarray(img_W)
    Wc[c.IMG:c.IMG + c.TXT] = 0.15 * np.asarray(txt_W)
    Wcatd = np.ascontiguousarray(
        Wc.astype(FP8).reshape(c.KP, 128, c.D).transpose(1, 0, 2)
    ).reshape(128, c.KP * c.D)

    # session weight chunks: rr col layout = emb(0:128) img(128:1128)
    # txt(1128:1896) ind(1896) pad(1897:1920)
    W2 = np.zeros((c.NW, 128, c.D), f32)
    rows = np.arange(128)
    for i, m in enumerate(range(1, 9)):
        gl = m * 128 + rows          # global rr col
        fi = gl - 128                # img feature index
        val = np.where((gl >= 128) & (gl < 1128), 1.0, 0.0)
        W2[i] = np.asarray(img_W)[np.clip(fi, 0, c.IMG - 1)] * val[:, None]
    W2[8, 104] = np.asarray(img_b)   # ind row (1896 = 14*128+104) -> bi
    for i, m in enumerate(range(8, 15)):
        gl = m * 128 + rows
        fi = gl - 1128
        val = np.where((gl >= 1128) & (gl < 1896), 1.0, 0.0)
        W2[9 + i] = np.asarray(txt_W)[np.clip(fi, 0, c.TXT - 1)] * val[:, None]
    W2[16, 104] = np.asarray(txt_b)
    Wsd = np.ascontiguousarray(
        W2.astype(BF16).transpose(1, 0, 2)).reshape(128, c.NW * c.D)

    def gather(tab, ids):
        r = tab[np.maximum(ids - 1, 0)]
        r[ids == 0] = 0.0
        return r

    in_maps = []
    for kk in range(c.NC):
        b0, b1 = kk * c.BS, (kk + 1) * c.BS
        # --- h0 projection inputs (pair-major entry order) ---
        iid = inputs[b0:b1].reshape(c.NPAIR, c.L2).ravel()
        gi = gather(img_table, iid)                     # [3200, 1000]
        gt = gather(txt_table, iid)                     # [3200, 768]
        A = np.zeros((c.KP * 128, c.NE), FP8)
        A[:c.IMG] = gi.T.astype(FP8)
        A[c.IMG:c.IMG + c.TXT] = gt.T.astype(FP8)
        prTd = np.ascontiguousarray(
            A.reshape(c.KP, 128, c.NE).transpose(1, 0, 2)
        ).reshape(128, c.KP * c.NE)
        ge = gather(emb_table, iid) + bcomb
        ge[iid == 0] = 0.0
        embTd = np.ascontiguousarray(ge.T.astype(BF16))

        # --- session raw rows (b-major entry order) ---
        tid = item[b0:b1].ravel()
        R = np.zeros((c.NE, c.RW), f32)
        re_ = gather(emb_table, tid) + bcomb
        re_[tid == 0] = 0.0
        R[:, 0:128] = re_
        R[:, 128:1128] = gather(img_table, tid)
        R[:, 1128:1896] = gather(txt_table, tid)
        R[:, 1896] = (tid > 0).astype(f32)
        rrd = np.ascontiguousarray(
            R.astype(FP8).reshape(c.KR, 128, c.RW).transpose(1, 0, 2)
        ).reshape(128, c.KR * c.RW)

        mk = maskf[b0:b1]
        M = np.zeros((c.NE, c.BS), f32)
        M[np.arange(c.NE), np.arange(c.NE) // c.L] = mk.ravel()
        mkTd = np.ascontiguousarray(
            M.astype(FP8).reshape(c.KR, 128, c.BS).transpose(1, 0, 2)
        ).reshape(128, c.KR * c.BS)

        # --- hypergraph blocks ---
        Hk = Hs[b0:b1]
        Gd = np.zeros((c.L2, c.NPAIR, c.L2), f32)
        GTd = np.zeros((c.L2, c.NPAIR, c.L2), f32)
        for p in range(c.NPAIR):
            Gd[:c.L, p, :c.L] = Hk[2 * p]
            Gd[c.L:, p, c.L:] = Hk[2 * p + 1]
            GTd[:c.L, p, :c.L] = Hk[2 * p].T
            GTd[c.L:, p, c.L:] = Hk[2 * p + 1].T
        seldm = np.zeros((c.BS, c.NPAIR, c.L2), f32)
        for p in range(c.NPAIR):
            seldm[2 * p, p, :c.L] = 1.0
            seldm[2 * p + 1, p, c.L:] = 1.0

        in_maps.append({
            "prTd": prTd, "embTd": embTd, "rrd": rrd, "mkTd": mkTd,
            "Wcatd": Wcatd, "Wsd": Wsd,
            "Gd": Gd.astype(BF16).reshape(c.L2, c.NPAIR * c.L2),
            "GTd": GTd.astype(BF16).reshape(c.L2, c.NPAIR * c.L2),
            "seld": seldm.astype(BF16).reshape(c.BS, c.NPAIR * c.L2),
            "mbld": np.ascontiguousarray(mk.T).astype(BF16),
            "gvW": np.asarray(gate_v_W).astype(f32),
            "gvB": np.asarray(gate_v_b).reshape(c.D, 1).astype(f32),
            "gtW": np.asarray(gate_t_W).astype(f32),
            "gtB": np.asarray(gate_t_b).reshape(c.D, 1).astype(f32),
            "q1W": np.asarray(qc_W1).astype(f32),
            "q1B": np.asarray(qc_b1).reshape(c.D, 1).astype(f32),
            "q2W": np.asarray(qc_W2).astype(f32),
        })
    return in_maps


def run(c: Cfg, trace=False, **inputs):
    nc = _get_program(c)
    in_maps = _prep_inputs(c, **{k: np.asarray(v) for k, v in inputs.items()})
    res = bass_utils.run_bass_kernel_spmd(
        nc, in_maps, core_ids=list(range(c.NC)), trace=trace)
    outs = []
    for r in res.results:
        o = r["outd"].reshape(c.L2, c.NPAIR, c.D).transpose(1, 0, 2)
        outs.append(o.reshape(c.NPAIR, 2, c.L, c.D).reshape(c.BS, c.L, c.D))
    out = np.concatenate(outs, axis=0)
    return out.astype(np.float32), res


def kernel(**inputs):
    out, _ = run(REAL, trace=False, **inputs)
    return out


# revision 7
# speedup vs baseline: 3.2751x; 1.2899x over previous
"""Trainium2 Bass kernel for the gnn_message_passing problem, v4.

Strategy (8 NeuronCores, SPMD, fully local — no collectives, no indirect DMA):
  - Each core handles 64 sessions.  The host stages, per core, the raw
    img/txt/emb rows referenced by its own `inputs` (3200 entries, fp8,
    transposed, row-chunk-blocked) and `item` (3200 entries, fp8,
    m-major-chunked) in exactly the SBUF layout the kernel wants, so every
    DMA is a handful of contiguous descriptors per partition and loads are
    split across both HWDGE queues (sync: projection, scalar: session) +
    gpsimd SWDGE for small constants.  DMA splits are along non-contraction
    axes so PE chains complete as quarters arrive.
  - h0 path: project input rows with weights-stationary fp8 matmuls
    (Wcat = [0.1*img_W; 0.15*txt_W; I] with emb' passed through the
    identity chunk), PE-transpose to pair-major rows.
  - Session path: masked per-session sums of raw item rows via
    rr-stationary fp8 matmuls producing transposed sums ST directly;
    projection via bf16 weight chunks (biases as weight rows against an
    indicator column).  Fusion math in [128, 64] layout; 2-way softmax
    computed as sigmoid of the logit difference.
  - Hypergraph: degrees via PE matmuls against ones + single batched
    reciprocals; deg_e folded into a pre-scaled GTn so e-steps evacuate
    with plain copies; groups of 4 pairs share PSUM banks so evacuations
    batch, and scale+add fuse into single vector ops.
"""

import sys

sys.path.insert(0, "/opt/trn_rl_repo")

import numpy as np
import ml_dtypes

import concourse.bass as bass
import concourse.bacc as bacc
import concourse.mybir as mybir
import concourse.tile as tile
from concourse import bass_utils

BF16 = ml_dtypes.bfloat16
FP8 = ml_dtypes.float8_e4m3fn


class Cfg:
    def __init__(self):
        self.N = 50000
        self.D = 128
        self.IMG = 1000
        self.TXT = 768
        self.B = 512
        self.L = 50
        self.NC = 8
        self.BS = self.B // self.NC      # 64 sessions per core
        self.NPAIR = self.BS // 2        # 32
        self.L2 = 2 * self.L             # 100
        self.NE = self.BS * self.L       # 3200 entries per core
        self.KP = 15                     # proj k-chunks (img|txt|pad|emb)
        self.KR = 25                     # rr row chunks (3200 = 25*128)
        self.RW = 1920                   # rr width (15*128): emb|img|txt|ind|pad
        self.MR = 15                     # rr col chunks
        self.NW = 17                     # session weight chunks
        # proj row-chunk blocks (entry dim)
        self.RCH = [(i * 512, 512) for i in range(6)] + [(3072, 128)]
        # free-dim offset of block rc in the prT tile
        self.RCOFF = [0]
        for _, w in self.RCH:
            self.RCOFF.append(self.RCOFF[-1] + self.KP * w)


REAL = Cfg()


def build_program(c: Cfg):
    f32 = mybir.dt.float32
    b16 = mybir.dt.bfloat16
    f8 = mybir.dt.float8e4
    AF = mybir.ActivationFunctionType
    OP = mybir.AluOpType

    nc = bacc.Bacc("TRN2", target_bir_lowering=False, debug=False,
                   num_devices=c.NC)

    def ein(nm, sh, dt):
        return nc.dram_tensor(nm, sh, dt, kind="ExternalInput")

    PRW = c.RCOFF[-1]                    # 15*3200 free elems
    prTd = ein("prTd", [128, PRW], f8)
    rrd = ein("rrd", [128, c.MR * c.KR * c.D], f8)
    mkTd = ein("mkTd", [128, c.KR * c.BS], f8)
    Wcatd = ein("Wcatd", [128, c.KP * c.D], f8)
    Wsd = ein("Wsd", [128, c.NW * c.D], b16)
    Gd = ein("Gd", [c.L2, c.NPAIR * c.L2], b16)
    GTd = ein("GTd", [c.L2, c.NPAIR * c.L2], b16)
    seld = ein("seld", [c.BS, c.NPAIR * c.L2], b16)
    mbld = ein("mbld", [c.L, c.BS], b16)
    gvW = ein("gvW", [c.D, c.D], f32)
    gvB = ein("gvB", [c.D, 1], f32)
    gtW = ein("gtW", [c.D, c.D], f32)
    gtB = ein("gtB", [c.D, 1], f32)
    q1W = ein("q1W", [c.D, c.D], f32)
    q1B = ein("q1B", [c.D, 1], f32)
    q2W = ein("q2W", [c.D, 1], f32)

    outd = nc.dram_tensor("outd", [c.L2, c.NPAIR * c.D], f32,
                          kind="ExternalOutput")

    IMCH = list(range(1, 9))     # img rows live in ST chunks 1..8
    TXCH = list(range(8, 15))    # txt rows live in ST chunks 8..14

    with tile.TileContext(nc) as tc:
        with (
            tc.tile_pool(name="wpool", bufs=1) as wp,
            tc.tile_pool(name="cpool", bufs=1) as cb,
            tc.tile_pool(name="cs", bufs=3) as cs,
        ):
            # ---------------- constants / identities ----------------
            identf = wp.tile([128, 128], f32, tag="idf")
            ident16 = wp.tile([128, 128], b16, tag="id16")
            from concourse.masks import make_identity
            make_identity(nc, identf[:])
            make_identity(nc, ident16[:])
            ones50 = wp.tile([c.L, 1], b16, tag="ones50")
            nc.vector.memset(ones50[:], 1.0)
            ones100 = wp.tile([c.L2, 1], b16, tag="ones100")
            nc.vector.memset(ones100[:], 1.0)
            onesf = wp.tile([1, c.D], f32, tag="onesf")
            nc.vector.memset(onesf[:], 1.0)

            # ---------------- gpsimd SWDGE small loads ----------------
            mbl = wp.tile([c.L, c.BS], b16, tag="mbl")
            nc.gpsimd.dma_start(mbl[:], mbld[:])
            mkT = wp.tile([128, c.KR * c.BS], f8, tag="mkT")
            nc.gpsimd.dma_start(mkT[:], mkTd[:])
            ws = wp.tile([128, c.NW * c.D], b16, tag="ws")
            nc.gpsimd.dma_start(ws[:], Wsd[:])
            wgv = cb.tile([c.D, c.D], f32, tag="wgv")
            wgt = cb.tile([c.D, c.D], f32, tag="wgt")
            wq1 = cb.tile([c.D, c.D], f32, tag="wq1")
            wq2 = cb.tile([c.D, 1], f32, tag="wq2")
            bgv = cb.tile([c.D, 1], f32, tag="bgv")
            bgt = cb.tile([c.D, 1], f32, tag="bgt")
            bq1 = cb.tile([c.D, 1], f32, tag="bq1")
            nc.gpsimd.dma_start(wgv[:], gvW[:])
            nc.gpsimd.dma_start(wgt[:], gtW[:])
            nc.gpsimd.dma_start(wq1[:], q1W[:])
            nc.gpsimd.dma_start(wq2[:], q2W[:])
            nc.gpsimd.dma_start(bgv[:], gvB[:])
            nc.gpsimd.dma_start(bgt[:], gtB[:])
            nc.gpsimd.dma_start(bq1[:], q1B[:])
            sel = wp.tile([c.BS, c.NPAIR * c.L2], b16, tag="sel")
            nc.gpsimd.dma_start(sel[:], seld[:])

            # ---------------- sync queue: projection inputs ----------------
            wc = wp.tile([128, c.KP * c.D], f8, tag="wc")
            nc.sync.dma_start(wc[:], Wcatd[:])
            prT = wp.tile([128, PRW], f8, tag="prT")
            for a, b in [(0, 2), (2, 4), (4, 6), (6, 7)]:
                nc.sync.dma_start(prT[:, c.RCOFF[a]:c.RCOFF[b]],
                                  prTd[:, c.RCOFF[a]:c.RCOFF[b]])

            # ---------------- scalar queue: session inputs ----------------
            rr = wp.tile([128, c.MR * c.KR * c.D], f8, tag="rr")
            MW = c.KR * c.D              # free elems per m block
            for a, b in [(0, 4), (4, 8), (8, 12), (12, 15)]:
                nc.scalar.dma_start(rr[:, a * MW:b * MW],
                                    rrd[:, a * MW:b * MW])
            Gsb = wp.tile([c.L2, c.NPAIR * c.L2], b16, tag="Gsb")
            nc.scalar.dma_start(Gsb[:], Gd[:])
            GTsb = wp.tile([c.L2, c.NPAIR * c.L2], b16, tag="GTsb")
            nc.scalar.dma_start(GTsb[:], GTd[:])

            psA_ctx = tc.tile_pool(name="psA", bufs=2, space="PSUM")
            psA = psA_ctx.__enter__()
            psS_ctx = tc.tile_pool(name="psS", bufs=2, space="PSUM")
            psS = psS_ctx.__enter__()
            psT_ctx = tc.tile_pool(name="psT", bufs=2, space="PSUM")
            psT = psT_ctx.__enter__()

            hTs = cb.tile([128, c.NE], b16, tag="hTs")
            ST = cb.tile([128, c.MR * c.BS], b16, tag="ST")
            Semb = cb.tile([128, c.BS], f32, tag="Semb")

            def proj_chain(rc):
                r0, w = c.RCH[rc]
                off = c.RCOFF[rc]
                pa = psA.tile([128, 512], f32, tag="psA", name=f"pa{rc}")
                for k in range(c.KP):
                    nc.tensor.matmul(
                        pa[:, 0:w], lhsT=wc[:, k * c.D:(k + 1) * c.D],
                        rhs=prT[:, off + k * w:off + (k + 1) * w],
                        start=(k == 0), stop=(k == c.KP - 1))
                nc.scalar.copy(hTs[:, r0:r0 + w], pa[:, 0:w])

            def sum_chain(m):
                ps = psS.tile([128, 512], f32, tag="psS", name=f"ps{m}")
                for k in range(c.KR):
                    nc.tensor.matmul(
                        ps[:, 0:c.BS],
                        lhsT=rr[:, (m * c.KR + k) * c.D:(m * c.KR + k + 1) * c.D],
                        rhs=mkT[:, k * c.BS:(k + 1) * c.BS],
                        start=(k == 0), stop=(k == c.KR - 1))
                if m == 0:
                    nc.vector.tensor_copy(Semb[:], ps[:, 0:c.BS])
                elif m % 2 == 0:
                    nc.vector.tensor_copy(ST[:, m * c.BS:(m + 1) * c.BS],
                                          ps[:, 0:c.BS])
                else:
                    nc.scalar.copy(ST[:, m * c.BS:(m + 1) * c.BS],
                                   ps[:, 0:c.BS])

            # interleave proj/sum chains to match DMA quarter arrivals
            order = ["p0", "m0", "m1", "p1", "m2", "m3", "p2", "m4", "m5",
                     "p3", "m6", "m7", "p4", "m8", "m9", "p5", "m10", "m11",
                     "p6", "m12", "m13", "m14"]
            for tk in order:
                if tk[0] == "p":
                    proj_chain(int(tk[1:]))
                else:
                    sum_chain(int(tk[1:]))

            # denom
            dT = psS.tile([128, 512], f32, tag="psS", name="dT")
            nc.tensor.matmul(dT[0:1, 0:c.BS], lhsT=ones50[:], rhs=mbl[:],
                             start=True, stop=True)
            invd = cb.tile([1, c.BS], f32, tag="invd")
            nc.vector.reciprocal(invd[:], dT[0:1, 0:c.BS])

            # h0 transposes (bf16)
            h0 = cb.tile([c.L2, c.NPAIR * c.D], b16, tag="h0")
            for p in range(c.NPAIR):
                tr = psT.tile([128, 512], b16, tag="psT16", name=f"tr{p}")
                nc.tensor.transpose(tr[0:c.L2, 0:c.D],
                                    hTs[:, p * c.L2:(p + 1) * c.L2],
                                    ident16[:])
                if p % 2 == 0:
                    nc.scalar.copy(h0[:, p * c.D:(p + 1) * c.D],
                                   tr[0:c.L2, 0:c.D])
                else:
                    nc.vector.tensor_copy(h0[:, p * c.D:(p + 1) * c.D],
                                          tr[0:c.L2, 0:c.D])

            # degrees on PE: deg_e = G.T @ 1, deg_n = GT.T @ 1
            pde = psT.tile([128, 512], f32, tag="psTf", name="pde")
            for p in range(c.NPAIR):
                nc.tensor.matmul(pde[0:c.L2, p:p + 1],
                                 lhsT=Gsb[:, p * c.L2:(p + 1) * c.L2],
                                 rhs=ones100[:], start=True, stop=True)
            ideA = cb.tile([c.L2, c.NPAIR], f32, tag="ideA")
            nc.vector.reciprocal(ideA[:], pde[0:c.L2, 0:c.NPAIR])
            pdn = psT.tile([128, 512], f32, tag="psTf", name="pdn")
            for p in range(c.NPAIR):
                nc.tensor.matmul(pdn[0:c.L2, p:p + 1],
                                 lhsT=GTsb[:, p * c.L2:(p + 1) * c.L2],
                                 rhs=ones100[:], start=True, stop=True)
            idnA = cb.tile([c.L2, c.NPAIR], f32, tag="idnA")
            nc.vector.reciprocal(idnA[:], pdn[0:c.L2, 0:c.NPAIR])

            # GTn = GT scaled by 1/deg_e along partitions (e)
            GTn = cb.tile([c.L2, c.NPAIR * c.L2], b16, tag="GTn")
            for p in range(c.NPAIR):
                nc.vector.tensor_scalar_mul(
                    GTn[:, p * c.L2:(p + 1) * c.L2],
                    GTsb[:, p * c.L2:(p + 1) * c.L2], ideA[:, p:p + 1])

            # ---------------- session projections ----------------
            pim = psA.tile([128, 512], f32, tag="psA", name="pim")
            for i, m in enumerate(IMCH):
                nc.tensor.matmul(pim[:, 0:c.BS],
                                 lhsT=ws[:, i * c.D:(i + 1) * c.D],
                                 rhs=ST[:, m * c.BS:(m + 1) * c.BS],
                                 start=(i == 0), stop=False)
            XimQ = cs.tile([c.D, c.BS], f32, tag="XimQ")
            nc.scalar.copy(XimQ[:], pim[:, 0:c.BS])
            nc.tensor.matmul(pim[:, 0:c.BS], lhsT=ws[:, 8 * c.D:9 * c.D],
                             rhs=ST[:, 14 * c.BS:15 * c.BS],
                             start=False, stop=True)
            Xim = cb.tile([c.D, c.BS], f32, tag="Xim")
            nc.vector.tensor_copy(Xim[:], pim[:, 0:c.BS])

            ptx = psA.tile([128, 512], f32, tag="psA", name="ptx")
            for i, m in enumerate(TXCH):
                nc.tensor.matmul(ptx[:, 0:c.BS],
                                 lhsT=ws[:, (9 + i) * c.D:(10 + i) * c.D],
                                 rhs=ST[:, m * c.BS:(m + 1) * c.BS],
                                 start=(i == 0), stop=False)
            XtxQ = cs.tile([c.D, c.BS], f32, tag="XtxQ")
            nc.scalar.copy(XtxQ[:], ptx[:, 0:c.BS])
            nc.tensor.matmul(ptx[:, 0:c.BS], lhsT=ws[:, 16 * c.D:17 * c.D],
                             rhs=ST[:, 14 * c.BS:15 * c.BS],
                             start=False, stop=True)
            Xtx = cb.tile([c.D, c.BS], f32, tag="Xtx")
            nc.vector.tensor_copy(Xtx[:], ptx[:, 0:c.BS])

            # Xit = Semb' + 0.1 Xim' + 0.15 Xtx'
            Xit = cb.tile([c.D, c.BS], f32, tag="Xit")
            nc.vector.scalar_tensor_tensor(Xit[:], XimQ[:], 0.1, Semb[:],
                                           op0=OP.mult, op1=OP.add)
            nc.vector.scalar_tensor_tensor(Xit[:], XtxQ[:], 0.15, Xit[:],
                                           op0=OP.mult, op1=OP.add)

            # ---------------- fusion (transposed [128, 64]) ----------------
            def rep_row(row, nm):
                rp = psA.tile([128, 512], f32, tag="psA", name=nm)
                nc.tensor.matmul(rp[:, 0:c.BS], lhsT=onesf[:], rhs=row,
                                 start=True, stop=True)
                return rp

            Xim_m = cb.tile([c.D, c.BS], f32, tag="Xim_m")
            Xtx_m = cb.tile([c.D, c.BS], f32, tag="Xtx_m")
            Xit_m = cb.tile([c.D, c.BS], f32, tag="Xit_m")
            ir = rep_row(invd[:], "ir")
            nc.vector.tensor_tensor(Xim_m[:], Xim[:], ir[:, 0:c.BS], op=OP.mult)
            nc.vector.tensor_tensor(Xtx_m[:], Xtx[:], ir[:, 0:c.BS], op=OP.mult)
            nc.vector.tensor_tensor(Xit_m[:], Xit[:], ir[:, 0:c.BS], op=OP.mult)

            pgv = psA.tile([128, 512], f32, tag="psA", name="pgv")
            nc.tensor.matmul(pgv[:, 0:c.BS], lhsT=wgv[:], rhs=Xim_m[:],
                             start=True, stop=True)
            gv1 = cs.tile([c.D, c.BS], f32, tag="gv1")
            nc.scalar.activation(gv1[:], pgv[:, 0:c.BS], AF.Sigmoid,
                                 bias=bgv[:, :1], scale=2.0)
            pgt = psA.tile([128, 512], f32, tag="psA", name="pgt")
            nc.tensor.matmul(pgt[:, 0:c.BS], lhsT=wgt[:], rhs=Xtx_m[:],
                             start=True, stop=True)
            gt1 = cs.tile([c.D, c.BS], f32, tag="gt1")
            nc.scalar.activation(gt1[:], pgt[:, 0:c.BS], AF.Sigmoid,
                                 bias=bgt[:, :1], scale=2.0)
            sid = cb.tile([c.D, c.BS], f32, tag="sid")
            std = cb.tile([c.D, c.BS], f32, tag="std")
            nc.vector.tensor_mul(sid[:], Xit_m[:], gv1[:])
            nc.vector.tensor_mul(std[:], Xit_m[:], gt1[:])

            def qc(xin, tag):
                pq = psA.tile([128, 512], f32, tag="psA", name="pq" + tag)
                nc.tensor.matmul(pq[:, 0:c.BS], lhsT=wq1[:], rhs=xin[:],
                                 start=True, stop=True)
                th = cs.tile([c.D, c.BS], f32, tag="th")
                nc.scalar.activation(th[:], pq[:, 0:c.BS], AF.Tanh,
                                     bias=bq1[:, :1], scale=1.0)
                qq = psS.tile([128, 512], f32, tag="psS", name="qq" + tag)
                nc.tensor.matmul(qq[0:1, 0:c.BS], lhsT=wq2[:], rhs=th[:],
                                 start=True, stop=True)
                qv = cs.tile([1, c.BS], f32, tag="qv" + tag)
                nc.vector.tensor_copy(qv[:], qq[0:1, 0:c.BS])
                return qv

            q1v = qc(sid, "a")
            q2v = qc(std, "b")
            # 2-way softmax: w1 = sigmoid(q1-q2), w2 = 1-w1
            qd = cs.tile([1, c.BS], f32, tag="qd")
            nc.vector.tensor_sub(qd[:], q1v[:], q2v[:])
            w1 = cs.tile([1, c.BS], f32, tag="w1")
            nc.scalar.activation(w1[:], qd[:], AF.Sigmoid)
            w1r = rep_row(w1[:], "w1r")
            # com = std + w1*(sid-std)
            com = cb.tile([c.D, c.BS], f32, tag="com")
            nc.vector.tensor_sub(com[:], sid[:], std[:])
            nc.vector.tensor_mul(com[:], com[:], w1r[:, 0:c.BS])
            nc.vector.tensor_add(com[:], com[:], std[:])

            pg2 = psA.tile([128, 512], f32, tag="psA", name="pg2")
            nc.tensor.matmul(pg2[:, 0:c.BS], lhsT=wgv[:], rhs=Xit_m[:],
                             start=True, stop=True)
            gv2 = cs.tile([c.D, c.BS], f32, tag="gv2")
            nc.scalar.activation(gv2[:], pg2[:, 0:c.BS], AF.Sigmoid,
                                 bias=bgv[:, :1], scale=1.0)
            pg3 = psA.tile([128, 512], f32, tag="psA", name="pg3")
            nc.tensor.matmul(pg3[:, 0:c.BS], lhsT=wgt[:], rhs=Xit_m[:],
                             start=True, stop=True)
            gt2 = cs.tile([c.D, c.BS], f32, tag="gt2")
            nc.scalar.activation(gt2[:], pg3[:, 0:c.BS], AF.Sigmoid,
                                 bias=bgt[:, :1], scale=1.0)

            sep = cs.tile([c.D, c.BS], f32, tag="sep")
            nc.vector.tensor_sub(sep[:], sid[:], com[:])
            nc.vector.tensor_mul(sep[:], gv2[:], sep[:])
            sep2 = cs.tile([c.D, c.BS], f32, tag="sep2")
            nc.vector.tensor_sub(sep2[:], std[:], com[:])
            nc.vector.tensor_mul(sep2[:], gt2[:], sep2[:])
            fus = cs.tile([c.D, c.BS], f32, tag="fus")
            nc.vector.tensor_add(fus[:], sep[:], sep2[:])
            nc.vector.tensor_add(fus[:], fus[:], com[:])
            Xs = cb.tile([c.D, c.BS], f32, tag="Xs")
            nc.vector.scalar_tensor_tensor(Xs[:], fus[:], 1.0 / 3.0, Xit_m[:],
                                           op0=OP.mult, op1=OP.add)
            nc.vector.tensor_add(Xs[:], Xs[:], Xim_m[:])
            nc.vector.tensor_add(Xs[:], Xs[:], Xtx_m[:])

            # transpose Xs -> XsT [64, 128] bf16
            trx = psT.tile([128, 512], f32, tag="psTf", name="trx")
            nc.tensor.transpose(trx[0:c.BS, 0:c.D], Xs[:], identf[:])
            XsT = cb.tile([c.BS, c.D], b16, tag="XsT")
            nc.vector.tensor_copy(XsT[:], trx[0:c.BS, 0:c.D])

            psT_ctx.__exit__(None, None, None)
            psS_ctx.__exit__(None, None, None)
            psA_ctx.__exit__(None, None, None)

            # ---------------- hypergraph, groups of 4 pairs ----------------
            houtf = cb.tile([c.L2, c.NPAIR * c.D], f32, tag="houtf")
            with (
                tc.tile_pool(name="psB", bufs=2, space="PSUM") as psB,
                tc.tile_pool(name="psE", bufs=2, space="PSUM") as psE,
                tc.tile_pool(name="psH", bufs=2, space="PSUM") as psH,
            ):
                for g in range(c.NPAIR // 4):
                    ps_ = [g * 4 + j for j in range(4)]
                    sb = psB.tile([128, 512], f32, tag="psB", name=f"sb{g}")
                    for j, p in enumerate(ps_):
                        nc.tensor.matmul(sb[0:c.L2, j * c.D:(j + 1) * c.D],
                                         lhsT=sel[:, p * c.L2:(p + 1) * c.L2],
                                         rhs=XsT[:], start=True, stop=True)
                    srepS = cs.tile([c.L2, 4 * c.D], f32, tag="srepS")
                    nc.scalar.copy(srepS[:], sb[0:c.L2, :])

                    pe1 = psE.tile([128, 512], f32, tag="psE", name=f"pe1{g}")
                    for j, p in enumerate(ps_):
                        nc.tensor.matmul(pe1[0:c.L2, j * c.D:(j + 1) * c.D],
                                         lhsT=Gsb[:, p * c.L2:(p + 1) * c.L2],
                                         rhs=h0[:, p * c.D:(p + 1) * c.D],
                                         start=True, stop=True)
                    ee1 = cs.tile([c.L2, 4 * c.D], b16, tag="ee1")
                    nc.scalar.copy(ee1[:], pe1[0:c.L2, :])

                    ph1 = psH.tile([128, 512], f32, tag="psH", name=f"ph1{g}")
                    for j, p in enumerate(ps_):
                        nc.tensor.matmul(ph1[0:c.L2, j * c.D:(j + 1) * c.D],
                                         lhsT=GTn[:, p * c.L2:(p + 1) * c.L2],
                                         rhs=ee1[:, j * c.D:(j + 1) * c.D],
                                         start=True, stop=True)
                    hh1 = cs.tile([c.L2, 4 * c.D], b16, tag="hh1")
                    for j, p in enumerate(ps_):
                        nc.vector.scalar_tensor_tensor(
                            hh1[:, j * c.D:(j + 1) * c.D],
                            ph1[0:c.L2, j * c.D:(j + 1) * c.D],
                            idnA[:, p:p + 1],
                            srepS[:, j * c.D:(j + 1) * c.D],
                            op0=OP.mult, op1=OP.add)

                    pe2 = psE.tile([128, 512], f32, tag="psE", name=f"pe2{g}")
                    for j, p in enumerate(ps_):
                        nc.tensor.matmul(pe2[0:c.L2, j * c.D:(j + 1) * c.D],
                                         lhsT=Gsb[:, p * c.L2:(p + 1) * c.L2],
                                         rhs=hh1[:, j * c.D:(j + 1) * c.D],
                                         start=True, stop=True)
                    ee2 = cs.tile([c.L2, 4 * c.D], b16, tag="ee2")
                    nc.scalar.copy(ee2[:], pe2[0:c.L2, :])

                    ph2 = psH.tile([128, 512], f32, tag="psH", name=f"ph2{g}")
                    for j, p in enumerate(ps_):
                        nc.tensor.matmul(ph2[0:c.L2, j * c.D:(j + 1) * c.D],
                                         lhsT=GTn[:, p * c.L2:(p + 1) * c.L2],
                                         rhs=ee2[:, j * c.D:(j + 1) * c.D],
                                         start=True, stop=True)
                    for j, p in enumerate(ps_):
                        nc.vector.scalar_tensor_tensor(
                            houtf[:, p * c.D:(p + 1) * c.D],
                            ph2[0:c.L2, j * c.D:(j + 1) * c.D],
                            idnA[:, p:p + 1],
                            srepS[:, j * c.D:(j + 1) * c.D],
                            op0=OP.mult, op1=OP.add)
                    nc.scalar.dma_start(
                        outd[:, g * 512:(g + 1) * 512],
                        houtf[:, g * 512:(g + 1) * 512])
    nc.compile()
    return nc


_CACHE = {}


def _get_program(c: Cfg):
    key = (c.N, c.B)
    if key not in _CACHE:
        _CACHE[key] = build_program(c)
    return _CACHE[key]


def _prep_inputs(c: Cfg, inputs, item, mask_item, Hs, emb_table, img_table,
                 txt_table, img_W, img_b, txt_W, txt_b, gate_v_W, gate_v_b,
                 gate_t_W, gate_t_b, qc_W1, qc_b1, qc_W2):
    f32 = np.float32
    inputs = np.asarray(inputs)
    item = np.asarray(item)
    maskf = np.asarray(mask_item).astype(f32)
    Hs = np.asarray(Hs).astype(f32)
    emb_table = np.asarray(emb_table).astype(f32)
    img_table = np.asarray(img_table).astype(f32)
    txt_table = np.asarray(txt_table).astype(f32)
    bcomb = (0.1 * np.asarray(img_b) + 0.15 * np.asarray(txt_b)).astype(f32)

    # Wcat: [0.1*img_W ; 0.15*txt_W ; pad ; I] -> [128, 15, 128] fp8
    Wc = np.zeros((c.KP * 128, c.D), f32)
    Wc[:c.IMG] = 0.1 * np.asarray(img_W)
    Wc[c.IMG:c.IMG + c.TXT] = 0.15 * np.asarray(txt_W)
    Wc[14 * 128:] = np.eye(c.D, dtype=f32)
    Wcatd = np.ascontiguousarray(
        Wc.astype(FP8).reshape(c.KP, 128, c.D).transpose(1, 0, 2)
    ).reshape(128, c.KP * c.D)

    # session weight chunks: rr col layout = emb(0:128) img(128:1128)
    # txt(1128:1896) ind(1896) pad(1897:1920)
    W2 = np.zeros((c.NW, 128, c.D), f32)
    rows = np.arange(128)
    for i, m in enumerate(range(1, 9)):
        gl = m * 128 + rows
        fi = gl - 128
        val = np.where((gl >= 128) & (gl < 1128), 1.0, 0.0)
        W2[i] = np.asarray(img_W)[np.clip(fi, 0, c.IMG - 1)] * val[:, None]
    W2[8, 104] = np.asarray(img_b)
    for i, m in enumerate(range(8, 15)):
        gl = m * 128 + rows
        fi = gl - 1128
        val = np.where((gl >= 1128) & (gl < 1896), 1.0, 0.0)
        W2[9 + i] = np.asarray(txt_W)[np.clip(fi, 0, c.TXT - 1)] * val[:, None]
    W2[16, 104] = np.asarray(txt_b)
    Wsd = np.ascontiguousarray(
        W2.astype(BF16).transpose(1, 0, 2)).reshape(128, c.NW * c.D)

    def gather(tab, ids):
        r = tab[np.maximum(ids - 1, 0)]
        r[ids == 0] = 0.0
        return r

    in_maps = []
    for kk in range(c.NC):
        b0, b1 = kk * c.BS, (kk + 1) * c.BS
        # --- h0 projection inputs (pair-major entry order) ---
        iid = inputs[b0:b1].reshape(c.NPAIR, c.L2).ravel()
        A = np.zeros((c.KP * 128, c.NE), FP8)
        A[:c.IMG] = gather(img_table, iid).T.astype(FP8)
        A[c.IMG:c.IMG + c.TXT] = gather(txt_table, iid).T.astype(FP8)
        ge = gather(emb_table, iid) + bcomb
        ge[iid == 0] = 0.0
        A[14 * 128:] = ge.T.astype(FP8)
        A3 = A.reshape(c.KP, 128, c.NE)
        blocks = [np.ascontiguousarray(
            A3[:, :, r0:r0 + w].transpose(1, 0, 2)).reshape(128, c.KP * w)
            for r0, w in c.RCH]
        prTd = np.concatenate(blocks, axis=1)

        # --- session raw rows (b-major entry order, m-major chunks) ---
        tid = item[b0:b1].ravel()
        R = np.zeros((c.NE, c.RW), f32)
        re_ = gather(emb_table, tid) + bcomb
        re_[tid == 0] = 0.0
        R[:, 0:128] = re_
        R[:, 128:1128] = gather(img_table, tid)
        R[:, 1128:1896] = gather(txt_table, tid)
        R[:, 1896] = (tid > 0).astype(f32)
        rrd = np.ascontiguousarray(
            R.astype(FP8).reshape(c.KR, 128, c.MR, c.D).transpose(1, 2, 0, 3)
        ).reshape(128, c.MR * c.KR * c.D)

        mk = maskf[b0:b1]
        M = np.zeros((c.NE, c.BS), f32)
        M[np.arange(c.NE), np.arange(c.NE) // c.L] = mk.ravel()
        mkTd = np.ascontiguousarray(
            M.astype(FP8).reshape(c.KR, 128, c.BS).transpose(1, 0, 2)
        ).reshape(128, c.KR * c.BS)

        # --- hypergraph blocks ---
        Hk = Hs[b0:b1]
        Gd = np.zeros((c.L2, c.NPAIR, c.L2), f32)
        GTd = np.zeros((c.L2, c.NPAIR, c.L2), f32)
        for p in range(c.NPAIR):
            Gd[:c.L, p, :c.L] = Hk[2 * p]
            Gd[c.L:, p, c.L:] = Hk[2 * p + 1]
            GTd[:c.L, p, :c.L] = Hk[2 * p].T
            GTd[c.L:, p, c.L:] = Hk[2 * p + 1].T
        seldm = np.zeros((c.BS, c.NPAIR, c.L2), f32)
        for p in range(c.NPAIR):
            seldm[2 * p, p, :c.L] = 1.0
            seldm[2 * p + 1, p, c.L:] = 1.0

        in_maps.append({
            "prTd": prTd, "rrd": rrd, "mkTd": mkTd,
            "Wcatd": Wcatd, "Wsd": Wsd,
            "Gd": Gd.astype(BF16).reshape(c.L2, c.NPAIR * c.L2),
            "GTd": GTd.astype(BF16).reshape(c.L2, c.NPAIR * c.L2),
            "seld": seldm.astype(BF16).reshape(c.BS, c.NPAIR * c.L2),
            "mbld": np.ascontiguousarray(mk.T).astype(BF16),
            "gvW": np.asarray(gate_v_W).astype(f32),
            "gvB": np.asarray(gate_v_b).reshape(c.D, 1).astype(f32),
            "gtW": np.asarray(gate_t_W).astype(f32),
            "gtB": np.asarray(gate_t_b).reshape(c.D, 1).astype(f32),
            "q1W": np.asarray(qc_W1).astype(f32),
            "q1B": np.asarray(qc_b1).reshape(c.D, 1).astype(f32),
            "q2W": np.asarray(qc_W2).astype(f32),
        })
    return in_maps


def run(c: Cfg, trace=False, **inputs):
    nc = _get_program(c)
    in_maps = _prep_inputs(c, **{k: np.asarray(v) for k, v in inputs.items()})
    res = bass_utils.run_bass_kernel_spmd(
        nc, in_maps, core_ids=list(range(c.NC)), trace=trace)
    outs = []
    for r in res.results:
        o = r["outd"].reshape(c.L2, c.NPAIR, c.D).transpose(1, 0, 2)
        outs.append(o.reshape(c.NPAIR, 2, c.L, c.D).reshape(c.BS, c.L, c.D))
    out = np.concatenate(outs, axis=0)
    return out.astype(np.float32), res


def kernel(**inputs):
    out, _ = run(REAL, trace=False, **inputs)
    return out


# revision 12
# speedup vs baseline: 3.5667x; 1.0891x over previous
"""Trainium2 Bass kernel for the gnn_message_passing problem, v4.

Strategy (8 NeuronCores, SPMD, fully local — no collectives, no indirect DMA):
  - Each core handles 64 sessions.  The host stages, per core, the raw
    img/txt/emb rows referenced by its own `inputs` (3200 entries, fp8,
    transposed, row-chunk-blocked) and `item` (3200 entries, fp8,
    m-major-chunked) in exactly the SBUF layout the kernel wants, so every
    DMA is a handful of contiguous descriptors per partition and loads are
    split across both HWDGE queues (sync: projection, scalar: session) +
    gpsimd SWDGE for small constants.  DMA splits are along non-contraction
    axes so PE chains complete as quarters arrive.
  - h0 path: project input rows with weights-stationary fp8 matmuls
    (Wcat = [0.1*img_W; 0.15*txt_W; I] with emb' passed through the
    identity chunk), PE-transpose to pair-major rows.
  - Session path: masked per-session sums of raw item rows via
    rr-stationary fp8 matmuls producing transposed sums ST directly;
    projection via bf16 weight chunks (biases as weight rows against an
    indicator column).  Fusion math in [128, 64] layout; 2-way softmax
    computed as sigmoid of the logit difference.
  - Hypergraph: degrees via PE matmuls against ones + single batched
    reciprocals; deg_e folded into a pre-scaled GTn so e-steps evacuate
    with plain copies; groups of 4 pairs share PSUM banks so evacuations
    batch, and scale+add fuse into single vector ops.
"""

import sys

sys.path.insert(0, "/opt/trn_rl_repo")

import numpy as np
import ml_dtypes

import concourse.bass as bass
import concourse.bacc as bacc
import concourse.mybir as mybir
import concourse.tile as tile
from concourse import bass_utils

BF16 = ml_dtypes.bfloat16
FP8 = ml_dtypes.float8_e4m3fn


class Cfg:
    def __init__(self):
        self.N = 50000
        self.D = 128
        self.IMG = 1000
        self.TXT = 768
        self.B = 512
        self.L = 50
        self.NC = 8
        self.BS = self.B // self.NC      # 64 sessions per core
        self.NPAIR = self.BS // 2        # 32
        self.L2 = 2 * self.L             # 100
        self.NE = self.BS * self.L       # 3200 entries per core
        self.KP = 15                     # proj k-chunks (img|txt|pad|emb)
        self.KR = 25                     # rr row chunks (3200 = 25*128)
        self.RW = 1920                   # rr width (15*128): emb|img|txt|ind|pad
        self.MR = 15                     # rr col chunks
        self.NW = 17                     # session weight chunks
        # proj row-chunk blocks (entry dim)
        self.RCH = [(i * 512, 512) for i in range(6)] + [(3072, 128)]
        # free-dim offset of block rc in the prT tile
        self.RCOFF = [0]
        for _, w in self.RCH:
            self.RCOFF.append(self.RCOFF[-1] + self.KP * w)


REAL = Cfg()


def build_program(c: Cfg):
    f32 = mybir.dt.float32
    b16 = mybir.dt.bfloat16
    f8 = mybir.dt.float8e4
    AF = mybir.ActivationFunctionType
    OP = mybir.AluOpType

    nc = bacc.Bacc("TRN2", target_bir_lowering=False, debug=False,
                   num_devices=c.NC)

    def ein(nm, sh, dt):
        return nc.dram_tensor(nm, sh, dt, kind="ExternalInput")

    PRW = c.RCOFF[-1]                    # 15*3200 free elems
    prTd = ein("prTd", [128, PRW], f8)
    rrd = ein("rrd", [128, c.MR * c.KR * c.D], f8)
    mkTd = ein("mkTd", [128, c.KR * c.BS], f8)
    Wcatd = ein("Wcatd", [128, c.KP * c.D], f8)
    Wsd = ein("Wsd", [128, c.NW * c.D], b16)
    Gd = ein("Gd", [c.L2, c.NPAIR * c.L2], b16)
    GTd = ein("GTd", [c.L2, c.NPAIR * c.L2], b16)
    seld = ein("seld", [c.BS, c.NPAIR * c.L2], b16)
    mbld = ein("mbld", [c.L, c.BS], b16)
    gvW = ein("gvW", [c.D, c.D], f32)
    gvB = ein("gvB", [c.D, 1], f32)
    gtW = ein("gtW", [c.D, c.D], f32)
    gtB = ein("gtB", [c.D, 1], f32)
    q1W = ein("q1W", [c.D, c.D], f32)
    q1B = ein("q1B", [c.D, 1], f32)
    q2W = ein("q2W", [c.D, 1], f32)

    outd = nc.dram_tensor("outd", [c.L2, c.NPAIR * c.D], f32,
                          kind="ExternalOutput")

    IMCH = list(range(1, 9))     # img rows live in ST chunks 1..8
    TXCH = list(range(8, 15))    # txt rows live in ST chunks 8..14

    with tile.TileContext(nc) as tc:
        with (
            tc.tile_pool(name="wpool", bufs=1) as wp,
            tc.tile_pool(name="cpool", bufs=1) as cb,
            tc.tile_pool(name="cs", bufs=3) as cs,
        ):
            # ---------------- constants / identities ----------------
            identf = wp.tile([128, 128], f32, tag="idf")
            ident16 = wp.tile([128, 128], b16, tag="id16")
            from concourse.masks import make_identity
            make_identity(nc, identf[:])
            make_identity(nc, ident16[:])
            ones50 = wp.tile([c.L, 1], b16, tag="ones50")
            nc.vector.memset(ones50[:], 1.0)
            ones100 = wp.tile([c.L2, 1], b16, tag="ones100")
            nc.vector.memset(ones100[:], 1.0)
            onesf = wp.tile([1, c.D], f32, tag="onesf")
            nc.vector.memset(onesf[:], 1.0)

            # ---------------- gpsimd SWDGE small loads ----------------
            mbl = wp.tile([c.L, c.BS], b16, tag="mbl")
            nc.gpsimd.dma_start(mbl[:], mbld[:])
            ws = wp.tile([128, c.NW * c.D], b16, tag="ws")
            nc.gpsimd.dma_start(ws[:], Wsd[:])
            wgv = cb.tile([c.D, c.D], f32, tag="wgv")
            wgt = cb.tile([c.D, c.D], f32, tag="wgt")
            wq1 = cb.tile([c.D, c.D], f32, tag="wq1")
            wq2 = cb.tile([c.D, 1], f32, tag="wq2")
            bgv = cb.tile([c.D, 1], f32, tag="bgv")
            bgt = cb.tile([c.D, 1], f32, tag="bgt")
            bq1 = cb.tile([c.D, 1], f32, tag="bq1")
            nc.gpsimd.dma_start(wgv[:], gvW[:])
            nc.gpsimd.dma_start(wgt[:], gtW[:])
            nc.gpsimd.dma_start(wq1[:], q1W[:])
            nc.gpsimd.dma_start(wq2[:], q2W[:])
            nc.gpsimd.dma_start(bgv[:], gvB[:])
            nc.gpsimd.dma_start(bgt[:], gtB[:])
            nc.gpsimd.dma_start(bq1[:], q1B[:])
            sel = wp.tile([c.BS, c.NPAIR * c.L2], b16, tag="sel")
            nc.gpsimd.dma_start(sel[:], seld[:])

            # ---------------- sync queue: projection inputs ----------------
            wc = wp.tile([128, c.KP * c.D], f8, tag="wc")
            nc.sync.dma_start(wc[:], Wcatd[:])
            prT = wp.tile([128, PRW], f8, tag="prT")
            for a, b in [(0, 1), (1, 3), (3, 5), (5, 7)]:
                nc.sync.dma_start(prT[:, c.RCOFF[a]:c.RCOFF[b]],
                                  prTd[:, c.RCOFF[a]:c.RCOFF[b]])
            Gsb = wp.tile([c.L2, c.NPAIR * c.L2], b16, tag="Gsb")
            nc.sync.dma_start(Gsb[:], Gd[:])
            GTsb = wp.tile([c.L2, c.NPAIR * c.L2], b16, tag="GTsb")
            nc.sync.dma_start(GTsb[:], GTd[:])

            # ---------------- scalar queue: session inputs ----------------
            mkT = wp.tile([128, c.KR * c.BS], f8, tag="mkT")
            nc.scalar.dma_start(mkT[:], mkTd[:])
            rr = wp.tile([128, c.MR * c.KR * c.D], f8, tag="rr")
            MW = c.KR * c.D              # free elems per m block
            for a, b in [(0, 4), (4, 8), (8, 12), (12, 15)]:
                nc.scalar.dma_start(rr[:, a * MW:b * MW],
                                    rrd[:, a * MW:b * MW])

            psA_ctx = tc.tile_pool(name="psA", bufs=2, space="PSUM")
            psA = psA_ctx.__enter__()
            psS_ctx = tc.tile_pool(name="psS", bufs=2, space="PSUM")
            psS = psS_ctx.__enter__()
            psT_ctx = tc.tile_pool(name="psT", bufs=2, space="PSUM")
            psT = psT_ctx.__enter__()

            hTs = cb.tile([128, c.NE], b16, tag="hTs")
            ST = cb.tile([128, c.MR * c.BS], b16, tag="ST")
            Semb = cb.tile([128, c.BS], f32, tag="Semb")

            DR = mybir.MatmulPerfMode.DoubleRow

            def proj_chain(rc):
                r0, w = c.RCH[rc]
                off = c.RCOFF[rc]
                pa = psA.tile([128, 512], f32, tag="psA", name=f"pa{rc}")
                for k2 in range(7):
                    nc.tensor.matmul(
                        pa[:, 0:w],
                        lhsT=wc[:, 2 * k2 * c.D:(2 * k2 + 2) * c.D].rearrange(
                            "p (k d) -> p k d", k=2),
                        rhs=prT[:, off + 2 * k2 * w:off + (2 * k2 + 2) * w]
                        .rearrange("p (k w) -> p k w", k=2),
                        start=(k2 == 0), stop=False, perf_mode=DR)
                nc.tensor.matmul(
                    pa[:, 0:w], lhsT=wc[:, 14 * c.D:15 * c.D],
                    rhs=prT[:, off + 14 * w:off + 15 * w],
                    start=False, stop=True)
                nc.scalar.copy(hTs[:, r0:r0 + w], pa[:, 0:w])

            def sum_chain(m):
                ps = psS.tile([128, 512], f32, tag="psS", name=f"ps{m}")
                o = m * c.KR * c.D
                for k2 in range(12):
                    nc.tensor.matmul(
                        ps[:, 0:c.BS],
                        lhsT=rr[:, o + 2 * k2 * c.D:o + (2 * k2 + 2) * c.D]
                        .rearrange("p (k d) -> p k d", k=2),
                        rhs=mkT[:, 2 * k2 * c.BS:(2 * k2 + 2) * c.BS]
                        .rearrange("p (k b) -> p k b", k=2),
                        start=(k2 == 0), stop=False, perf_mode=DR)
                nc.tensor.matmul(
                    ps[:, 0:c.BS], lhsT=rr[:, o + 24 * c.D:o + 25 * c.D],
                    rhs=mkT[:, 24 * c.BS:25 * c.BS], start=False, stop=True)
                if m == 0:
                    nc.vector.tensor_copy(Semb[:], ps[:, 0:c.BS])
                elif m % 2 == 0:
                    nc.vector.tensor_copy(ST[:, m * c.BS:(m + 1) * c.BS],
                                          ps[:, 0:c.BS])
                else:
                    nc.scalar.copy(ST[:, m * c.BS:(m + 1) * c.BS],
                                   ps[:, 0:c.BS])

            h0 = cb.tile([c.L2, c.NPAIR * c.D], b16, tag="h0")

            def transp(p):
                tr = psT.tile([128, 512], b16, tag="psT16", name=f"tr{p}")
                nc.tensor.transpose(tr[0:c.L2, 0:c.D],
                                    hTs[:, p * c.L2:(p + 1) * c.L2],
                                    ident16[:])
                if p % 2 == 0:
                    nc.scalar.copy(h0[:, p * c.D:(p + 1) * c.D],
                                   tr[0:c.L2, 0:c.D])
                else:
                    nc.vector.tensor_copy(h0[:, p * c.D:(p + 1) * c.D],
                                          tr[0:c.L2, 0:c.D])

            # interleave proj/sum chains to match DMA quarter arrivals;
            # transposes fill the gap while the last rr quarter lands
            order = (["p0", "m0", "m1", "p1", "m2", "m3", "p2", "m4", "m5",
                      "p3", "m6", "m7", "p4", "m8", "m9", "p5", "m10", "m11",
                      "p6"]
                     + [f"t{p}" for p in range(c.NPAIR)]
                     + ["m12", "m13", "m14"])
            for tk in order:
                if tk[0] == "p":
                    proj_chain(int(tk[1:]))
                elif tk[0] == "t":
                    transp(int(tk[1:]))
                else:
                    sum_chain(int(tk[1:]))

            # denom
            dT = psS.tile([128, 512], f32, tag="psS", name="dT")
            nc.tensor.matmul(dT[0:1, 0:c.BS], lhsT=ones50[:], rhs=mbl[:],
                             start=True, stop=True)
            invd = cb.tile([1, c.BS], f32, tag="invd")
            nc.vector.reciprocal(invd[:], dT[0:1, 0:c.BS])

            # degrees on PE: deg_e = G.T @ 1, deg_n = GT.T @ 1
            pde = psT.tile([128, 512], f32, tag="psTf", name="pde")
            for p in range(c.NPAIR):
                nc.tensor.matmul(pde[0:c.L2, p:p + 1],
                                 lhsT=Gsb[:, p * c.L2:(p + 1) * c.L2],
                                 rhs=ones100[:], start=True, stop=True)
            ideA = cb.tile([c.L2, c.NPAIR], f32, tag="ideA")
            nc.vector.reciprocal(ideA[:], pde[0:c.L2, 0:c.NPAIR])
            pdn = psT.tile([128, 512], f32, tag="psTf", name="pdn")
            for p in range(c.NPAIR):
                nc.tensor.matmul(pdn[0:c.L2, p:p + 1],
                                 lhsT=GTsb[:, p * c.L2:(p + 1) * c.L2],
                                 rhs=ones100[:], start=True, stop=True)
            idnA = cb.tile([c.L2, c.NPAIR], f32, tag="idnA")
            nc.vector.reciprocal(idnA[:], pdn[0:c.L2, 0:c.NPAIR])

            # GTn = GT scaled by 1/deg_e along partitions (e)
            GTn = cb.tile([c.L2, c.NPAIR * c.L2], b16, tag="GTn")
            for p in range(c.NPAIR):
                nc.vector.tensor_scalar_mul(
                    GTn[:, p * c.L2:(p + 1) * c.L2],
                    GTsb[:, p * c.L2:(p + 1) * c.L2], ideA[:, p:p + 1])

            # ---------------- session projections ----------------
            pim = psA.tile([128, 512], f32, tag="psA", name="pim")
            for i, m in enumerate(IMCH):
                nc.tensor.matmul(pim[:, 0:c.BS],
                                 lhsT=ws[:, i * c.D:(i + 1) * c.D],
                                 rhs=ST[:, m * c.BS:(m + 1) * c.BS],
                                 start=(i == 0), stop=False)
            XimQ = cs.tile([c.D, c.BS], f32, tag="XimQ")
            nc.scalar.copy(XimQ[:], pim[:, 0:c.BS])
            nc.tensor.matmul(pim[:, 0:c.BS], lhsT=ws[:, 8 * c.D:9 * c.D],
                             rhs=ST[:, 14 * c.BS:15 * c.BS],
                             start=False, stop=True)
            Xim = cb.tile([c.D, c.BS], f32, tag="Xim")
            nc.vector.tensor_copy(Xim[:], pim[:, 0:c.BS])

            ptx = psA.tile([128, 512], f32, tag="psA", name="ptx")
            for i, m in enumerate(TXCH):
                nc.tensor.matmul(ptx[:, 0:c.BS],
                                 lhsT=ws[:, (9 + i) * c.D:(10 + i) * c.D],
                                 rhs=ST[:, m * c.BS:(m + 1) * c.BS],
                                 start=(i == 0), stop=False)
            XtxQ = cs.tile([c.D, c.BS], f32, tag="XtxQ")
            nc.scalar.copy(XtxQ[:], ptx[:, 0:c.BS])
            nc.tensor.matmul(ptx[:, 0:c.BS], lhsT=ws[:, 16 * c.D:17 * c.D],
                             rhs=ST[:, 14 * c.BS:15 * c.BS],
                             start=False, stop=True)
            Xtx = cb.tile([c.D, c.BS], f32, tag="Xtx")
            nc.vector.tensor_copy(Xtx[:], ptx[:, 0:c.BS])

            # Xit = Semb' + 0.1 Xim' + 0.15 Xtx'
            Xit = cb.tile([c.D, c.BS], f32, tag="Xit")
            nc.vector.scalar_tensor_tensor(Xit[:], XimQ[:], 0.1, Semb[:],
                                           op0=OP.mult, op1=OP.add)
            nc.vector.scalar_tensor_tensor(Xit[:], XtxQ[:], 0.15, Xit[:],
                                           op0=OP.mult, op1=OP.add)

            # ---------------- fusion (transposed [128, 64]) ----------------
            def rep_row(row, nm):
                rp = psA.tile([128, 512], f32, tag="psA", name=nm)
                nc.tensor.matmul(rp[:, 0:c.BS], lhsT=onesf[:], rhs=row,
                                 start=True, stop=True)
                return rp

            Xim_m = cb.tile([c.D, c.BS], f32, tag="Xim_m")
            Xtx_m = cb.tile([c.D, c.BS], f32, tag="Xtx_m")
            Xit_m = cb.tile([c.D, c.BS], f32, tag="Xit_m")
            ir = rep_row(invd[:], "ir")
            nc.vector.tensor_tensor(Xim_m[:], Xim[:], ir[:, 0:c.BS], op=OP.mult)
            nc.vector.tensor_tensor(Xtx_m[:], Xtx[:], ir[:, 0:c.BS], op=OP.mult)
            nc.vector.tensor_tensor(Xit_m[:], Xit[:], ir[:, 0:c.BS], op=OP.mult)

            pgv = psA.tile([128, 512], f32, tag="psA", name="pgv")
            nc.tensor.matmul(pgv[:, 0:c.BS], lhsT=wgv[:], rhs=Xim_m[:],
                             start=True, stop=True)
            gv1 = cs.tile([c.D, c.BS], f32, tag="gv1")
            nc.scalar.activation(gv1[:], pgv[:, 0:c.BS], AF.Sigmoid,
                                 bias=bgv[:, :1], scale=2.0)
            pgt = psA.tile([128, 512], f32, tag="psA", name="pgt")
            nc.tensor.matmul(pgt[:, 0:c.BS], lhsT=wgt[:], rhs=Xtx_m[:],
                             start=True, stop=True)
            gt1 = cs.tile([c.D, c.BS], f32, tag="gt1")
            nc.scalar.activation(gt1[:], pgt[:, 0:c.BS], AF.Sigmoid,
                                 bias=bgt[:, :1], scale=2.0)
            sid = cb.tile([c.D, c.BS], f32, tag="sid")
            std = cb.tile([c.D, c.BS], f32, tag="std")
            nc.vector.tensor_mul(sid[:], Xit_m[:], gv1[:])
            nc.vector.tensor_mul(std[:], Xit_m[:], gt1[:])

            def qc(xin, tag):
                pq = psA.tile([128, 512], f32, tag="psA", name="pq" + tag)
                nc.tensor.matmul(pq[:, 0:c.BS], lhsT=wq1[:], rhs=xin[:],
                                 start=True, stop=True)
                th = cs.tile([c.D, c.BS], f32, tag="th")
                nc.scalar.activation(th[:], pq[:, 0:c.BS], AF.Tanh,
                                     bias=bq1[:, :1], scale=1.0)
                qq = psS.tile([128, 512], f32, tag="psS", name="qq" + tag)
                nc.tensor.matmul(qq[0:1, 0:c.BS], lhsT=wq2[:], rhs=th[:],
                                 start=True, stop=True)
                qv = cs.tile([1, c.BS], f32, tag="qv" + tag)
                nc.vector.tensor_copy(qv[:], qq[0:1, 0:c.BS])
                return qv

            q1v = qc(sid, "a")
            q2v = qc(std, "b")
            # 2-way softmax: w1 = sigmoid(q1-q2), w2 = 1-w1
            qd = cs.tile([1, c.BS], f32, tag="qd")
            nc.vector.tensor_sub(qd[:], q1v[:], q2v[:])
            w1 = cs.tile([1, c.BS], f32, tag="w1")
            nc.scalar.activation(w1[:], qd[:], AF.Sigmoid)
            w1r = rep_row(w1[:], "w1r")
            # com = std + w1*(sid-std)
            com = cb.tile([c.D, c.BS], f32, tag="com")
            nc.vector.tensor_sub(com[:], sid[:], std[:])
            nc.vector.tensor_mul(com[:], com[:], w1r[:, 0:c.BS])
            nc.vector.tensor_add(com[:], com[:], std[:])

            pg2 = psA.tile([128, 512], f32, tag="psA", name="pg2")
            nc.tensor.matmul(pg2[:, 0:c.BS], lhsT=wgv[:], rhs=Xit_m[:],
                             start=True, stop=True)
            gv2 = cs.tile([c.D, c.BS], f32, tag="gv2")
            nc.scalar.activation(gv2[:], pg2[:, 0:c.BS], AF.Sigmoid,
                                 bias=bgv[:, :1], scale=1.0)
            pg3 = psA.tile([128, 512], f32, tag="psA", name="pg3")
            nc.tensor.matmul(pg3[:, 0:c.BS], lhsT=wgt[:], rhs=Xit_m[:],
                             start=True, stop=True)
            gt2 = cs.tile([c.D, c.BS], f32, tag="gt2")
            nc.scalar.activation(gt2[:], pg3[:, 0:c.BS], AF.Sigmoid,
                                 bias=bgt[:, :1], scale=1.0)

            sep = cs.tile([c.D, c.BS], f32, tag="sep")
            nc.vector.tensor_sub(sep[:], sid[:], com[:])
            nc.vector.tensor_mul(sep[:], gv2[:], sep[:])
            sep2 = cs.tile([c.D, c.BS], f32, tag="sep2")
            nc.vector.tensor_sub(sep2[:], std[:], com[:])
            nc.vector.tensor_mul(sep2[:], gt2[:], sep2[:])
            fus = cs.tile([c.D, c.BS], f32, tag="fus")
            nc.vector.tensor_add(fus[:], sep[:], sep2[:])
            nc.vector.tensor_add(fus[:], fus[:], com[:])
            Xs = cb.tile([c.D, c.BS], f32, tag="Xs")
            nc.vector.scalar_tensor_tensor(Xs[:], fus[:], 1.0 / 3.0, Xit_m[:],
                                           op0=OP.mult, op1=OP.add)
            nc.vector.tensor_add(Xs[:], Xs[:], Xim_m[:])
            nc.vector.tensor_add(Xs[:], Xs[:], Xtx_m[:])

            # transpose Xs -> XsT [64, 128] bf16
            trx = psT.tile([128, 512], f32, tag="psTf", name="trx")
            nc.tensor.transpose(trx[0:c.BS, 0:c.D], Xs[:], identf[:])
            XsT = cb.tile([c.BS, c.D], b16, tag="XsT")
            nc.vector.tensor_copy(XsT[:], trx[0:c.BS, 0:c.D])

            psT_ctx.__exit__(None, None, None)
            psS_ctx.__exit__(None, None, None)
            psA_ctx.__exit__(None, None, None)

            # ---------------- hypergraph, groups of 4 pairs ----------------
            houtf = cb.tile([c.L2, c.NPAIR * c.D], f32, tag="houtf")
            with (
                tc.tile_pool(name="psB", bufs=2, space="PSUM") as psB,
                tc.tile_pool(name="psE", bufs=2, space="PSUM") as psE,
                tc.tile_pool(name="psH", bufs=2, space="PSUM") as psH,
            ):
                for g in range(c.NPAIR // 4):
                    ps_ = [g * 4 + j for j in range(4)]
                    sb = psB.tile([128, 512], f32, tag="psB", name=f"sb{g}")
                    for j, p in enumerate(ps_):
                        nc.tensor.matmul(sb[0:c.L2, j * c.D:(j + 1) * c.D],
                                         lhsT=sel[:, p * c.L2:(p + 1) * c.L2],
                                         rhs=XsT[:], start=True, stop=True)
                    srepS = cs.tile([c.L2, 4 * c.D], f32, tag="srepS")
                    nc.scalar.copy(srepS[:], sb[0:c.L2, :])

                    pe1 = psE.tile([128, 512], f32, tag="psE", name=f"pe1{g}")
                    for j, p in enumerate(ps_):
                        nc.tensor.matmul(pe1[0:c.L2, j * c.D:(j + 1) * c.D],
                                         lhsT=Gsb[:, p * c.L2:(p + 1) * c.L2],
                                         rhs=h0[:, p * c.D:(p + 1) * c.D],
                                         start=True, stop=True)
                    ee1 = cs.tile([c.L2, 4 * c.D], b16, tag="ee1")
                    nc.scalar.copy(ee1[:], pe1[0:c.L2, :])

                    ph1 = psH.tile([128, 512], f32, tag="psH", name=f"ph1{g}")
                    for j, p in enumerate(ps_):
                        nc.tensor.matmul(ph1[0:c.L2, j * c.D:(j + 1) * c.D],
                                         lhsT=GTn[:, p * c.L2:(p + 1) * c.L2],
                                         rhs=ee1[:, j * c.D:(j + 1) * c.D],
                                         start=True, stop=True)
                    hh1 = cs.tile([c.L2, 4 * c.D], b16, tag="hh1")
                    for j, p in enumerate(ps_):
                        nc.vector.scalar_tensor_tensor(
                            hh1[:, j * c.D:(j + 1) * c.D],
                            ph1[0:c.L2, j * c.D:(j + 1) * c.D],
                            idnA[:, p:p + 1],
                            srepS[:, j * c.D:(j + 1) * c.D],
                            op0=OP.mult, op1=OP.add)

                    pe2 = psE.tile([128, 512], f32, tag="psE", name=f"pe2{g}")
                    for j, p in enumerate(ps_):
                        nc.tensor.matmul(pe2[0:c.L2, j * c.D:(j + 1) * c.D],
                                         lhsT=Gsb[:, p * c.L2:(p + 1) * c.L2],
                                         rhs=hh1[:, j * c.D:(j + 1) * c.D],
                                         start=True, stop=True)
                    ee2 = cs.tile([c.L2, 4 * c.D], b16, tag="ee2")
                    nc.vector.tensor_copy(ee2[:], pe2[0:c.L2, :])

                    ph2 = psH.tile([128, 512], f32, tag="psH", name=f"ph2{g}")
                    for j, p in enumerate(ps_):
                        nc.tensor.matmul(ph2[0:c.L2, j * c.D:(j + 1) * c.D],
                                         lhsT=GTn[:, p * c.L2:(p + 1) * c.L2],
                                         rhs=ee2[:, j * c.D:(j + 1) * c.D],
                                         start=True, stop=True)
                    for j, p in enumerate(ps_):
                        nc.vector.scalar_tensor_tensor(
                            houtf[:, p * c.D:(p + 1) * c.D],
                            ph2[0:c.L2, j * c.D:(j + 1) * c.D],
                            idnA[:, p:p + 1],
                            srepS[:, j * c.D:(j + 1) * c.D],
                            op0=OP.mult, op1=OP.add)
                    nc.scalar.dma_start(
                        outd[:, g * 512:(g + 1) * 512],
                        houtf[:, g * 512:(g + 1) * 512])
    nc.compile()
    return nc


_CACHE = {}


def _get_program(c: Cfg):
    key = (c.N, c.B)
    if key not in _CACHE:
        _CACHE[key] = build_program(c)
    return _CACHE[key]


def _prep_inputs(c: Cfg, inputs, item, mask_item, Hs, emb_table, img_table,
                 txt_table, img_W, img_b, txt_W, txt_b, gate_v_W, gate_v_b,
                 gate_t_W, gate_t_b, qc_W1, qc_b1, qc_W2):
    f32 = np.float32
    inputs = np.asarray(inputs)
    item = np.asarray(item)
    maskf = np.asarray(mask_item).astype(f32)
    Hs = np.asarray(Hs).astype(f32)
    emb_table = np.asarray(emb_table).astype(f32)
    img_table = np.asarray(img_table).astype(f32)
    txt_table = np.asarray(txt_table).astype(f32)
    bcomb = (0.1 * np.asarray(img_b) + 0.15 * np.asarray(txt_b)).astype(f32)

    # Wcat: [0.1*img_W ; 0.15*txt_W ; pad ; I] -> [128, 15, 128] fp8
    Wc = np.zeros((c.KP * 128, c.D), f32)
    Wc[:c.IMG] = 0.1 * np.asarray(img_W)
    Wc[c.IMG:c.IMG + c.TXT] = 0.15 * np.asarray(txt_W)
    Wc[14 * 128:] = np.eye(c.D, dtype=f32)
    Wcatd = np.ascontiguousarray(
        Wc.astype(FP8).reshape(c.KP, 128, c.D).transpose(1, 0, 2)
    ).reshape(128, c.KP * c.D)

    # session weight chunks: rr col layout = emb(0:128) img(128:1128)
    # txt(1128:1896) ind(1896) pad(1897:1920)
    W2 = np.zeros((c.NW, 128, c.D), f32)
    rows = np.arange(128)
    for i, m in enumerate(range(1, 9)):
        gl = m * 128 + rows
        fi = gl - 128
        val = np.where((gl >= 128) & (gl < 1128), 1.0, 0.0)
        W2[i] = np.asarray(img_W)[np.clip(fi, 0, c.IMG - 1)] * val[:, None]
    W2[8, 104] = np.asarray(img_b)
    for i, m in enumerate(range(8, 15)):
        gl = m * 128 + rows
        fi = gl - 1128
        val = np.where((gl >= 1128) & (gl < 1896), 1.0, 0.0)
        W2[9 + i] = np.asarray(txt_W)[np.clip(fi, 0, c.TXT - 1)] * val[:, None]
    W2[16, 104] = np.asarray(txt_b)
    Wsd = np.ascontiguousarray(
        W2.astype(BF16).transpose(1, 0, 2)).reshape(128, c.NW * c.D)

    def gather(tab, ids):
        r = tab[np.maximum(ids - 1, 0)]
        r[ids == 0] = 0.0
        return r

    in_maps = []
    for kk in range(c.NC):
        b0, b1 = kk * c.BS, (kk + 1) * c.BS
        # --- h0 projection inputs (pair-major entry order) ---
        iid = inputs[b0:b1].reshape(c.NPAIR, c.L2).ravel()
        A = np.zeros((c.KP * 128, c.NE), FP8)
        A[:c.IMG] = gather(img_table, iid).T.astype(FP8)
        A[c.IMG:c.IMG + c.TXT] = gather(txt_table, iid).T.astype(FP8)
        ge = gather(emb_table, iid) + bcomb
        ge[iid == 0] = 0.0
        A[14 * 128:] = ge.T.astype(FP8)
        A3 = A.reshape(c.KP, 128, c.NE)
        blocks = [np.ascontiguousarray(
            A3[:, :, r0:r0 + w].transpose(1, 0, 2)).reshape(128, c.KP * w)
            for r0, w in c.RCH]
        prTd = np.concatenate(blocks, axis=1)

        # --- session raw rows (b-major entry order, m-major chunks) ---
        tid = item[b0:b1].ravel()
        R = np.zeros((c.NE, c.RW), f32)
        re_ = gather(emb_table, tid) + bcomb
        re_[tid == 0] = 0.0
        R[:, 0:128] = re_
        R[:, 128:1128] = gather(img_table, tid)
        R[:, 1128:1896] = gather(txt_table, tid)
        R[:, 1896] = (tid > 0).astype(f32)
        rrd = np.ascontiguousarray(
            R.astype(FP8).reshape(c.KR, 128, c.MR, c.D).transpose(1, 2, 0, 3)
        ).reshape(128, c.MR * c.KR * c.D)

        mk = maskf[b0:b1]
        M = np.zeros((c.NE, c.BS), f32)
        M[np.arange(c.NE), np.arange(c.NE) // c.L] = mk.ravel()
        mkTd = np.ascontiguousarray(
            M.astype(FP8).reshape(c.KR, 128, c.BS).transpose(1, 0, 2)
        ).reshape(128, c.KR * c.BS)

        # --- hypergraph blocks ---
        Hk = Hs[b0:b1]
        Gd = np.zeros((c.L2, c.NPAIR, c.L2), f32)
        GTd = np.zeros((c.L2, c.NPAIR, c.L2), f32)
        for p in range(c.NPAIR):
            Gd[:c.L, p, :c.L] = Hk[2 * p]
            Gd[c.L:, p, c.L:] = Hk[2 * p + 1]
            GTd[:c.L, p, :c.L] = Hk[2 * p].T
            GTd[c.L:, p, c.L:] = Hk[2 * p + 1].T
        seldm = np.zeros((c.BS, c.NPAIR, c.L2), f32)
        for p in range(c.NPAIR):
            seldm[2 * p, p, :c.L] = 1.0
            seldm[2 * p + 1, p, c.L:] = 1.0

        in_maps.append({
            "prTd": prTd, "rrd": rrd, "mkTd": mkTd,
            "Wcatd": Wcatd, "Wsd": Wsd,
            "Gd": Gd.astype(BF16).reshape(c.L2, c.NPAIR * c.L2),
            "GTd": GTd.astype(BF16).reshape(c.L2, c.NPAIR * c.L2),
            "seld": seldm.astype(BF16).reshape(c.BS, c.NPAIR * c.L2),
            "mbld": np.ascontiguousarray(mk.T).astype(BF16),
            "gvW": np.asarray(gate_v_W).astype(f32),
            "gvB": np.asarray(gate_v_b).reshape(c.D, 1).astype(f32),
            "gtW": np.asarray(gate_t_W).astype(f32),
            "gtB": np.asarray(gate_t_b).reshape(c.D, 1).astype(f32),
            "q1W": np.asarray(qc_W1).astype(f32),
            "q1B": np.asarray(qc_b1).reshape(c.D, 1).astype(f32),
            "q2W": np.asarray(qc_W2).astype(f32),
        })
    return in_maps


def run(c: Cfg, trace=False, **inputs):
    nc = _get_program(c)
    in_maps = _prep_inputs(c, **{k: np.asarray(v) for k, v in inputs.items()})
    res = bass_utils.run_bass_kernel_spmd(
        nc, in_maps, core_ids=list(range(c.NC)), trace=trace)
    outs = []
    for r in res.results:
        o = r["outd"].reshape(c.L2, c.NPAIR, c.D).transpose(1, 0, 2)
        outs.append(o.reshape(c.NPAIR, 2, c.L, c.D).reshape(c.BS, c.L, c.D))
    out = np.concatenate(outs, axis=0)
    return out.astype(np.float32), res


def kernel(**inputs):
    out, _ = run(REAL, trace=False, **inputs)
    return out


# revision 19
# speedup vs baseline: 4.1899x; 1.1747x over previous
"""Trainium2 Bass kernel for the gnn_message_passing problem, v4.

Strategy (8 NeuronCores, SPMD, fully local — no collectives, no indirect DMA):
  - Each core handles 64 sessions.  The host stages, per core, the raw
    img/txt/emb rows referenced by its own `inputs` (3200 entries, fp8,
    transposed, row-chunk-blocked) and `item` (3200 entries, fp8,
    m-major-chunked) in exactly the SBUF layout the kernel wants, so every
    DMA is a handful of contiguous descriptors per partition and loads are
    split across both HWDGE queues (sync: projection, scalar: session) +
    gpsimd SWDGE for small constants.  DMA splits are along non-contraction
    axes so PE chains complete as quarters arrive.
  - h0 path: project input rows with weights-stationary fp8 matmuls
    (Wcat = [0.1*img_W; 0.15*txt_W; I] with emb' passed through the
    identity chunk), PE-transpose to pair-major rows.
  - Session path: masked per-session sums of raw item rows via
    rr-stationary fp8 matmuls producing transposed sums ST directly;
    projection via bf16 weight chunks (biases as weight rows against an
    indicator column).  Fusion math in [128, 64] layout; 2-way softmax
    computed as sigmoid of the logit difference.
  - Hypergraph: degrees via PE matmuls against ones + single batched
    reciprocals; deg_e folded into a pre-scaled GTn so e-steps evacuate
    with plain copies; groups of 4 pairs share PSUM banks so evacuations
    batch, and scale+add fuse into single vector ops.
"""

import sys

sys.path.insert(0, "/opt/trn_rl_repo")

import numpy as np
import ml_dtypes

import concourse.bass as bass
import concourse.bacc as bacc
import concourse.mybir as mybir
import concourse.tile as tile
from concourse import bass_utils

BF16 = ml_dtypes.bfloat16
FP8 = ml_dtypes.float8_e4m3fn


class Cfg:
    def __init__(self):
        self.N = 50000
        self.D = 128
        self.IMG = 1000
        self.TXT = 768
        self.B = 512
        self.L = 50
        self.NC = 8
        self.BS = self.B // self.NC      # 64 sessions per core
        self.NPAIR = self.BS // 2        # 32
        self.L2 = 2 * self.L             # 100
        self.NE = self.BS * self.L       # 3200 entries per core
        self.KP = 15                     # proj k-chunks (img|txt|pad|emb)
        self.KR = 25                     # rr row chunks (3200 = 25*128)
        self.RW = 1920                   # rr width (15*128): emb|img|txt|ind|pad
        self.MR = 15                     # rr col chunks
        self.NW = 17                     # session weight chunks
        # proj row-chunk blocks (entry dim)
        self.RCH = [(i * 512, 512) for i in range(6)] + [(3072, 128)]
        # free-dim offset of block rc in the prT tile
        self.RCOFF = [0]
        for _, w in self.RCH:
            self.RCOFF.append(self.RCOFF[-1] + self.KP * w)


REAL = Cfg()


def build_program(c: Cfg):
    f32 = mybir.dt.float32
    b16 = mybir.dt.bfloat16
    f8 = mybir.dt.float8e4
    AF = mybir.ActivationFunctionType
    OP = mybir.AluOpType

    nc = bacc.Bacc("TRN2", target_bir_lowering=False, debug=False,
                   num_devices=c.NC)

    def ein(nm, sh, dt):
        return nc.dram_tensor(nm, sh, dt, kind="ExternalInput")

    PRW = c.RCOFF[-1]                    # 15*3200 free elems
    prTd = ein("prTd", [128, PRW], f8)
    rrd = ein("rrd", [128, c.MR * c.KR * c.D], f8)
    mkTd = ein("mkTd", [128, c.KR * c.BS], f8)
    Wcatd = ein("Wcatd", [128, c.KP * c.D], f8)
    Wsd = ein("Wsd", [128, c.NW * c.D], b16)
    Gd = ein("Gd", [c.L2, c.NPAIR * c.L2], f8)
    GTd = ein("GTd", [c.L2, c.NPAIR * c.L2], f8)
    seld = ein("seld", [c.BS, c.NPAIR * c.L2], b16)
    mbld = ein("mbld", [c.L, c.BS], b16)
    gvW = ein("gvW", [c.D, c.D], f32)
    gvB = ein("gvB", [c.D, 1], f32)
    gtW = ein("gtW", [c.D, c.D], f32)
    gtB = ein("gtB", [c.D, 1], f32)
    q1W = ein("q1W", [c.D, c.D], f32)
    q1B = ein("q1B", [c.D, 1], f32)
    q2W = ein("q2W", [c.D, 1], f32)

    f16 = mybir.dt.float16
    outd = nc.dram_tensor("outd", [c.L2, c.NPAIR * c.D], f16,
                          kind="ExternalOutput")

    IMCH = list(range(1, 9))     # img rows live in ST chunks 1..8
    TXCH = list(range(8, 15))    # txt rows live in ST chunks 8..14

    with tile.TileContext(nc) as tc:
        with (
            tc.tile_pool(name="wpool", bufs=1) as wp,
            tc.tile_pool(name="cpool", bufs=1) as cb,
            tc.tile_pool(name="cs", bufs=3) as cs,
        ):
            # ---------------- constants / identities ----------------
            identf = wp.tile([128, 128], f32, tag="idf")
            ident16 = wp.tile([128, 128], b16, tag="id16")
            from concourse.masks import make_identity
            make_identity(nc, identf[:])
            make_identity(nc, ident16[:])
            ones50 = wp.tile([c.L, 1], b16, tag="ones50")
            nc.vector.memset(ones50[:], 1.0)
            ones100 = wp.tile([c.L2, 1], b16, tag="ones100")
            nc.vector.memset(ones100[:], 1.0)
            onesf = wp.tile([1, c.D], f32, tag="onesf")
            nc.vector.memset(onesf[:], 1.0)

            # ---------------- gpsimd SWDGE small loads ----------------
            mbl = wp.tile([c.L, c.BS], b16, tag="mbl")
            nc.gpsimd.dma_start(mbl[:], mbld[:])
            ws = wp.tile([128, c.NW * c.D], b16, tag="ws")
            nc.gpsimd.dma_start(ws[:], Wsd[:])
            wgv = cb.tile([c.D, c.D], f32, tag="wgv")
            wgt = cb.tile([c.D, c.D], f32, tag="wgt")
            wq1 = cb.tile([c.D, c.D], f32, tag="wq1")
            wq2 = cb.tile([c.D, 1], f32, tag="wq2")
            bgv = cb.tile([c.D, 1], f32, tag="bgv")
            bgt = cb.tile([c.D, 1], f32, tag="bgt")
            bq1 = cb.tile([c.D, 1], f32, tag="bq1")
            nc.gpsimd.dma_start(wgv[:], gvW[:])
            nc.gpsimd.dma_start(wgt[:], gtW[:])
            nc.gpsimd.dma_start(wq1[:], q1W[:])
            nc.gpsimd.dma_start(wq2[:], q2W[:])
            nc.gpsimd.dma_start(bgv[:], gvB[:])
            nc.gpsimd.dma_start(bgt[:], gtB[:])
            nc.gpsimd.dma_start(bq1[:], q1B[:])
            # ---------------- sync queue: projection inputs ----------------
            wc = wp.tile([128, c.KP * c.D], f8, tag="wc")
            nc.sync.dma_start(wc[:], Wcatd[:])
            prT = wp.tile([128, PRW], f8, tag="prT")
            nc.sync.dma_start(prT[:, c.RCOFF[0]:c.RCOFF[1]],
                              prTd[:, c.RCOFF[0]:c.RCOFF[1]])
            Gsb = wp.tile([c.L2, c.NPAIR * c.L2], f8, tag="Gsb")
            nc.sync.dma_start(Gsb[:], Gd[:])
            GTsb = wp.tile([c.L2, c.NPAIR * c.L2], f8, tag="GTsb")
            nc.sync.dma_start(GTsb[:], GTd[:])
            for a, b in [(1, 3), (3, 5), (5, 7)]:
                nc.sync.dma_start(prT[:, c.RCOFF[a]:c.RCOFF[b]],
                                  prTd[:, c.RCOFF[a]:c.RCOFF[b]])

            # ---------------- scalar queue: session inputs ----------------
            mkT = wp.tile([128, c.KR * c.BS], f8, tag="mkT")
            nc.scalar.dma_start(mkT[:], mkTd[:])
            rr = wp.tile([128, c.MR * c.KR * c.D], f8, tag="rr")
            MW = c.KR * c.D              # free elems per m block
            for a, b in [(0, 4), (4, 8), (8, 12), (12, 15)]:
                nc.scalar.dma_start(rr[:, a * MW:b * MW],
                                    rrd[:, a * MW:b * MW])
            sel = wp.tile([c.BS, c.NPAIR * c.L2], b16, tag="sel")
            nc.scalar.dma_start(sel[:], seld[:])

            psA_ctx = tc.tile_pool(name="psA", bufs=2, space="PSUM")
            psA = psA_ctx.__enter__()
            psS_ctx = tc.tile_pool(name="psS", bufs=2, space="PSUM")
            psS = psS_ctx.__enter__()
            psT_ctx = tc.tile_pool(name="psT", bufs=2, space="PSUM")
            psT = psT_ctx.__enter__()

            hTs = cb.tile([128, c.NE], b16, tag="hTs")
            ST = cb.tile([128, c.MR * c.BS], b16, tag="ST")
            Semb = cb.tile([128, c.BS], f32, tag="Semb")

            DR = mybir.MatmulPerfMode.DoubleRow

            def proj_chain(rc):
                r0, w = c.RCH[rc]
                off = c.RCOFF[rc]
                pa = psA.tile([128, 512], f32, tag="psA", name=f"pa{rc}")
                for k2 in range(7):
                    nc.tensor.matmul(
                        pa[:, 0:w],
                        lhsT=wc[:, 2 * k2 * c.D:(2 * k2 + 2) * c.D].rearrange(
                            "p (k d) -> p k d", k=2),
                        rhs=prT[:, off + 2 * k2 * w:off + (2 * k2 + 2) * w]
                        .rearrange("p (k w) -> p k w", k=2),
                        start=(k2 == 0), stop=False, perf_mode=DR)
                nc.tensor.matmul(
                    pa[:, 0:w], lhsT=wc[:, 14 * c.D:15 * c.D],
                    rhs=prT[:, off + 14 * w:off + 15 * w],
                    start=False, stop=True)
                nc.scalar.copy(hTs[:, r0:r0 + w], pa[:, 0:w])

            def sum_chain(m):
                ps = psS.tile([128, 512], f32, tag="psS", name=f"ps{m}")
                o = m * c.KR * c.D
                for k2 in range(12):
                    nc.tensor.matmul(
                        ps[:, 0:c.BS],
                        lhsT=rr[:, o + 2 * k2 * c.D:o + (2 * k2 + 2) * c.D]
                        .rearrange("p (k d) -> p k d", k=2),
                        rhs=mkT[:, 2 * k2 * c.BS:(2 * k2 + 2) * c.BS]
                        .rearrange("p (k b) -> p k b", k=2),
                        start=(k2 == 0), stop=False, perf_mode=DR)
                nc.tensor.matmul(
                    ps[:, 0:c.BS], lhsT=rr[:, o + 24 * c.D:o + 25 * c.D],
                    rhs=mkT[:, 24 * c.BS:25 * c.BS], start=False, stop=True)
                if m == 0:
                    nc.vector.tensor_copy(Semb[:], ps[:, 0:c.BS])
                elif m % 2 == 0:
                    nc.vector.tensor_copy(ST[:, m * c.BS:(m + 1) * c.BS],
                                          ps[:, 0:c.BS])
                else:
                    nc.scalar.copy(ST[:, m * c.BS:(m + 1) * c.BS],
                                   ps[:, 0:c.BS])

            h0 = cb.tile([c.L2, c.NPAIR * c.D], b16, tag="h0")

            def transp(p):
                tr = psT.tile([128, 512], b16, tag="psT16", name=f"tr{p}")
                nc.tensor.transpose(tr[0:c.L2, 0:c.D],
                                    hTs[:, p * c.L2:(p + 1) * c.L2],
                                    ident16[:])
                if p % 2 == 0:
                    nc.scalar.copy(h0[:, p * c.D:(p + 1) * c.D],
                                   tr[0:c.L2, 0:c.D])
                else:
                    nc.vector.tensor_copy(h0[:, p * c.D:(p + 1) * c.D],
                                          tr[0:c.L2, 0:c.D])

            # degrees on PE + GTn + Abar build helpers
            ideA = cb.tile([c.L2, c.NPAIR], f32, tag="ideA")
            idnA = cb.tile([c.L2, c.NPAIR], f32, tag="idnA")
            GTn = cb.tile([c.L2, c.NPAIR * c.L2], b16, tag="GTn")
            Asb = cb.tile([c.L2, c.NPAIR * c.L2], b16, tag="Asb")

            def deg_all():
                pde = psT.tile([128, 512], f32, tag="psTf", name="pde")
                for p in range(c.NPAIR):
                    nc.tensor.matmul(pde[0:c.L2, p:p + 1],
                                     lhsT=Gsb[:, p * c.L2:(p + 1) * c.L2],
                                     rhs=ones100[:], start=True, stop=True)
                nc.vector.reciprocal(ideA[:], pde[0:c.L2, 0:c.NPAIR])
                pdn = psT.tile([128, 512], f32, tag="psTf", name="pdn")
                for p in range(c.NPAIR):
                    nc.tensor.matmul(pdn[0:c.L2, p:p + 1],
                                     lhsT=GTsb[:, p * c.L2:(p + 1) * c.L2],
                                     rhs=ones100[:], start=True, stop=True)
                nc.vector.reciprocal(idnA[:], pdn[0:c.L2, 0:c.NPAIR])

            def gtn_all():
                # GTn = GT scaled by 1/deg_e along partitions (e)
                for p in range(c.NPAIR):
                    nc.vector.tensor_scalar_mul(
                        GTn[:, p * c.L2:(p + 1) * c.L2],
                        GTsb[:, p * c.L2:(p + 1) * c.L2], ideA[:, p:p + 1])

            def abar(g):
                # Abar = H De^-1 H^T (symmetric), 4 pairs per PSUM tile
                pa_ = psT.tile([128, 512], f32, tag="psTf", name=f"ab{g}")
                for j in range(4):
                    p = 4 * g + j
                    nc.tensor.matmul(pa_[0:c.L2, j * c.L2:(j + 1) * c.L2],
                                     lhsT=GTn[:, p * c.L2:(p + 1) * c.L2],
                                     rhs=GTsb[:, p * c.L2:(p + 1) * c.L2],
                                     start=True, stop=True)
                if g % 2 == 0:
                    nc.scalar.copy(Asb[:, g * 4 * c.L2:(g + 1) * 4 * c.L2],
                                   pa_[0:c.L2, 0:4 * c.L2])
                else:
                    nc.vector.tensor_copy(
                        Asb[:, g * 4 * c.L2:(g + 1) * 4 * c.L2],
                        pa_[0:c.L2, 0:4 * c.L2])

            # interleave proj/sum chains with deg/GTn/Abar prep and h0
            # transposes to match DMA arrivals and fill PE gaps
            for tk in (["p0", "m0", "m1", "p1", "m2", "m3", "deg", "p2",
                        "m4", "m5", "gtn", "p3", "m6", "m7", "a0", "a1",
                        "a2", "a3", "p4", "m8", "m9", "a4", "a5", "a6", "a7",
                        "p5", "m10", "m11", "p6"]
                       + [f"t{p}" for p in range(c.NPAIR)]
                       + ["m12", "m13", "m14"]):
                if tk == "deg":
                    deg_all()
                elif tk == "gtn":
                    gtn_all()
                elif tk[0] == "a":
                    abar(int(tk[1:]))
                elif tk[0] == "p":
                    proj_chain(int(tk[1:]))
                elif tk[0] == "t":
                    transp(int(tk[1:]))
                else:
                    sum_chain(int(tk[1:]))

            # denom
            dT = psS.tile([128, 512], f32, tag="psS", name="dT")
            nc.tensor.matmul(dT[0:1, 0:c.BS], lhsT=ones50[:], rhs=mbl[:],
                             start=True, stop=True)
            invd = cb.tile([1, c.BS], f32, tag="invd")
            nc.vector.reciprocal(invd[:], dT[0:1, 0:c.BS])

            # ---------------- session projections ----------------
            pim = psA.tile([128, 512], f32, tag="psA", name="pim")
            for i, m in enumerate(IMCH):
                nc.tensor.matmul(pim[:, 0:c.BS],
                                 lhsT=ws[:, i * c.D:(i + 1) * c.D],
                                 rhs=ST[:, m * c.BS:(m + 1) * c.BS],
                                 start=(i == 0), stop=False)
            XimQ = cs.tile([c.D, c.BS], f32, tag="XimQ")
            nc.scalar.copy(XimQ[:], pim[:, 0:c.BS])
            nc.tensor.matmul(pim[:, 0:c.BS], lhsT=ws[:, 8 * c.D:9 * c.D],
                             rhs=ST[:, 14 * c.BS:15 * c.BS],
                             start=False, stop=True)
            Xim = cb.tile([c.D, c.BS], f32, tag="Xim")
            nc.vector.tensor_copy(Xim[:], pim[:, 0:c.BS])

            ptx = psA.tile([128, 512], f32, tag="psA", name="ptx")
            for i, m in enumerate(TXCH):
                nc.tensor.matmul(ptx[:, 0:c.BS],
                                 lhsT=ws[:, (9 + i) * c.D:(10 + i) * c.D],
                                 rhs=ST[:, m * c.BS:(m + 1) * c.BS],
                                 start=(i == 0), stop=False)
            XtxQ = cs.tile([c.D, c.BS], f32, tag="XtxQ")
            nc.scalar.copy(XtxQ[:], ptx[:, 0:c.BS])
            nc.tensor.matmul(ptx[:, 0:c.BS], lhsT=ws[:, 16 * c.D:17 * c.D],
                             rhs=ST[:, 14 * c.BS:15 * c.BS],
                             start=False, stop=True)
            Xtx = cb.tile([c.D, c.BS], f32, tag="Xtx")
            nc.vector.tensor_copy(Xtx[:], ptx[:, 0:c.BS])

            # Xit = Semb' + 0.1 Xim' + 0.15 Xtx'
            Xit = cb.tile([c.D, c.BS], f32, tag="Xit")
            nc.vector.scalar_tensor_tensor(Xit[:], XimQ[:], 0.1, Semb[:],
                                           op0=OP.mult, op1=OP.add)
            nc.vector.scalar_tensor_tensor(Xit[:], XtxQ[:], 0.15, Xit[:],
                                           op0=OP.mult, op1=OP.add)

            # ---------------- fusion (transposed [128, 64]) ----------------
            def rep_row(row, nm):
                rp = psA.tile([128, 512], f32, tag="psA", name=nm)
                nc.tensor.matmul(rp[:, 0:c.BS], lhsT=onesf[:], rhs=row,
                                 start=True, stop=True)
                return rp

            Xim_m = cb.tile([c.D, c.BS], f32, tag="Xim_m")
            Xtx_m = cb.tile([c.D, c.BS], f32, tag="Xtx_m")
            Xit_m = cb.tile([c.D, c.BS], f32, tag="Xit_m")
            ir = rep_row(invd[:], "ir")
            nc.vector.tensor_tensor(Xim_m[:], Xim[:], ir[:, 0:c.BS], op=OP.mult)
            nc.vector.tensor_tensor(Xtx_m[:], Xtx[:], ir[:, 0:c.BS], op=OP.mult)
            nc.vector.tensor_tensor(Xit_m[:], Xit[:], ir[:, 0:c.BS], op=OP.mult)

            pgv = psA.tile([128, 512], f32, tag="psA", name="pgv")
            nc.tensor.matmul(pgv[:, 0:c.BS], lhsT=wgv[:], rhs=Xim_m[:],
                             start=True, stop=True)
            gv1 = cs.tile([c.D, c.BS], f32, tag="gv1")
            nc.scalar.activation(gv1[:], pgv[:, 0:c.BS], AF.Sigmoid,
                                 bias=bgv[:, :1], scale=2.0)
            pgt = psA.tile([128, 512], f32, tag="psA", name="pgt")
            nc.tensor.matmul(pgt[:, 0:c.BS], lhsT=wgt[:], rhs=Xtx_m[:],
                             start=True, stop=True)
            gt1 = cs.tile([c.D, c.BS], f32, tag="gt1")
            nc.scalar.activation(gt1[:], pgt[:, 0:c.BS], AF.Sigmoid,
                                 bias=bgt[:, :1], scale=2.0)
            sid = cb.tile([c.D, c.BS], f32, tag="sid")
            std = cb.tile([c.D, c.BS], f32, tag="std")
            nc.vector.tensor_mul(sid[:], Xit_m[:], gv1[:])
            nc.vector.tensor_mul(std[:], Xit_m[:], gt1[:])

            def qc(xin, tag):
                pq = psA.tile([128, 512], f32, tag="psA", name="pq" + tag)
                nc.tensor.matmul(pq[:, 0:c.BS], lhsT=wq1[:], rhs=xin[:],
                                 start=True, stop=True)
                th = cs.tile([c.D, c.BS], f32, tag="th")
                nc.scalar.activation(th[:], pq[:, 0:c.BS], AF.Tanh,
                                     bias=bq1[:, :1], scale=1.0)
                qq = psS.tile([128, 512], f32, tag="psS", name="qq" + tag)
                nc.tensor.matmul(qq[0:1, 0:c.BS], lhsT=wq2[:], rhs=th[:],
                                 start=True, stop=True)
                qv = cs.tile([1, c.BS], f32, tag="qv" + tag)
                nc.vector.tensor_copy(qv[:], qq[0:1, 0:c.BS])
                return qv

            q1v = qc(sid, "a")
            q2v = qc(std, "b")
            # 2-way softmax: w1 = sigmoid(q1-q2), w2 = 1-w1
            qd = cs.tile([1, c.BS], f32, tag="qd")
            nc.vector.tensor_sub(qd[:], q1v[:], q2v[:])
            w1 = cs.tile([1, c.BS], f32, tag="w1")
            nc.scalar.activation(w1[:], qd[:], AF.Sigmoid)
            w1r = rep_row(w1[:], "w1r")
            # com = std + w1*(sid-std)
            com = cb.tile([c.D, c.BS], f32, tag="com")
            nc.vector.tensor_sub(com[:], sid[:], std[:])
            nc.vector.tensor_mul(com[:], com[:], w1r[:, 0:c.BS])
            nc.vector.tensor_add(com[:], com[:], std[:])

            pg2 = psA.tile([128, 512], f32, tag="psA", name="pg2")
            nc.tensor.matmul(pg2[:, 0:c.BS], lhsT=wgv[:], rhs=Xit_m[:],
                             start=True, stop=True)
            gv2 = cs.tile([c.D, c.BS], f32, tag="gv2")
            nc.scalar.activation(gv2[:], pg2[:, 0:c.BS], AF.Sigmoid,
                                 bias=bgv[:, :1], scale=1.0)
            pg3 = psA.tile([128, 512], f32, tag="psA", name="pg3")
            nc.tensor.matmul(pg3[:, 0:c.BS], lhsT=wgt[:], rhs=Xit_m[:],
                             start=True, stop=True)
            gt2 = cs.tile([c.D, c.BS], f32, tag="gt2")
            nc.scalar.activation(gt2[:], pg3[:, 0:c.BS], AF.Sigmoid,
                                 bias=bgt[:, :1], scale=1.0)

            sep = cs.tile([c.D, c.BS], f32, tag="sep")
            nc.vector.tensor_sub(sep[:], sid[:], com[:])
            nc.vector.tensor_mul(sep[:], gv2[:], sep[:])
            sep2 = cs.tile([c.D, c.BS], f32, tag="sep2")
            nc.vector.tensor_sub(sep2[:], std[:], com[:])
            nc.vector.tensor_mul(sep2[:], gt2[:], sep2[:])
            fus = cs.tile([c.D, c.BS], f32, tag="fus")
            nc.vector.tensor_add(fus[:], sep[:], sep2[:])
            nc.vector.tensor_add(fus[:], fus[:], com[:])
            Xs = cb.tile([c.D, c.BS], f32, tag="Xs")
            nc.vector.scalar_tensor_tensor(Xs[:], fus[:], 1.0 / 3.0, Xit_m[:],
                                           op0=OP.mult, op1=OP.add)
            nc.vector.tensor_add(Xs[:], Xs[:], Xim_m[:])
            nc.vector.tensor_add(Xs[:], Xs[:], Xtx_m[:])

            # transpose Xs -> XsT [64, 128] bf16
            trx = psT.tile([128, 512], f32, tag="psTf", name="trx")
            nc.tensor.transpose(trx[0:c.BS, 0:c.D], Xs[:], identf[:])
            XsT = cb.tile([c.BS, c.D], b16, tag="XsT")
            nc.vector.tensor_copy(XsT[:], trx[0:c.BS, 0:c.D])

            psT_ctx.__exit__(None, None, None)
            psS_ctx.__exit__(None, None, None)
            psA_ctx.__exit__(None, None, None)

            # ---------------- hypergraph, groups of 4 pairs ----------------
            # h1 = Abar h0 / deg_n + srep ; h2 = Abar h1 / deg_n + srep
            houtf = cb.tile([c.L2, c.NPAIR * c.D], f16, tag="houtf")
            with (
                tc.tile_pool(name="psB", bufs=2, space="PSUM") as psB,
                tc.tile_pool(name="psH", bufs=5, space="PSUM") as psH,
            ):
                for g in range(c.NPAIR // 4):
                    ps_ = [g * 4 + j for j in range(4)]
                    sb = psB.tile([128, 512], f32, tag="psB", name=f"sb{g}")
                    for j, p in enumerate(ps_):
                        nc.tensor.matmul(sb[0:c.L2, j * c.D:(j + 1) * c.D],
                                         lhsT=sel[:, p * c.L2:(p + 1) * c.L2],
                                         rhs=XsT[:], start=True, stop=True)
                    srepS = cs.tile([c.L2, 4 * c.D], f32, tag="srepS")
                    if g % 2 == 0:
                        nc.scalar.copy(srepS[:], sb[0:c.L2, :])
                    else:
                        nc.vector.tensor_copy(srepS[:], sb[0:c.L2, :])

                    ph1 = psH.tile([128, 512], f32, tag="psH", name=f"ph1{g}")
                    for j, p in enumerate(ps_):
                        nc.tensor.matmul(ph1[0:c.L2, j * c.D:(j + 1) * c.D],
                                         lhsT=Asb[:, p * c.L2:(p + 1) * c.L2],
                                         rhs=h0[:, p * c.D:(p + 1) * c.D],
                                         start=True, stop=True)
                    hh1 = cs.tile([c.L2, 4 * c.D], b16, tag="hh1")
                    for j, p in enumerate(ps_):
                        nc.vector.scalar_tensor_tensor(
                            hh1[:, j * c.D:(j + 1) * c.D],
                            ph1[0:c.L2, j * c.D:(j + 1) * c.D],
                            idnA[:, p:p + 1],
                            srepS[:, j * c.D:(j + 1) * c.D],
                            op0=OP.mult, op1=OP.add)

                    ph2 = psH.tile([128, 512], f32, tag="psH", name=f"ph2{g}")
                    for j, p in enumerate(ps_):
                        nc.tensor.matmul(ph2[0:c.L2, j * c.D:(j + 1) * c.D],
                                         lhsT=Asb[:, p * c.L2:(p + 1) * c.L2],
                                         rhs=hh1[:, j * c.D:(j + 1) * c.D],
                                         start=True, stop=True)
                    for j, p in enumerate(ps_):
                        nc.vector.scalar_tensor_tensor(
                            houtf[:, p * c.D:(p + 1) * c.D],
                            ph2[0:c.L2, j * c.D:(j + 1) * c.D],
                            idnA[:, p:p + 1],
                            srepS[:, j * c.D:(j + 1) * c.D],
                            op0=OP.mult, op1=OP.add)
                    nc.scalar.dma_start(
                        outd[:, g * 512:(g + 1) * 512],
                        houtf[:, g * 512:(g + 1) * 512])
    nc.compile()
    return nc


_CACHE = {}


def _get_program(c: Cfg):
    key = (c.N, c.B)
    if key not in _CACHE:
        _CACHE[key] = build_program(c)
    return _CACHE[key]


def _prep_inputs(c: Cfg, inputs, item, mask_item, Hs, emb_table, img_table,
                 txt_table, img_W, img_b, txt_W, txt_b, gate_v_W, gate_v_b,
                 gate_t_W, gate_t_b, qc_W1, qc_b1, qc_W2):
    f32 = np.float32
    inputs = np.asarray(inputs)
    item = np.asarray(item)
    maskf = np.asarray(mask_item).astype(f32)
    Hs = np.asarray(Hs).astype(f32)
    emb_table = np.asarray(emb_table).astype(f32)
    img_table = np.asarray(img_table).astype(f32)
    txt_table = np.asarray(txt_table).astype(f32)
    bcomb = (0.1 * np.asarray(img_b) + 0.15 * np.asarray(txt_b)).astype(f32)

    # Wcat: [0.1*img_W ; 0.15*txt_W ; pad ; I] -> [128, 15, 128] fp8
    Wc = np.zeros((c.KP * 128, c.D), f32)
    Wc[:c.IMG] = 0.1 * np.asarray(img_W)
    Wc[c.IMG:c.IMG + c.TXT] = 0.15 * np.asarray(txt_W)
    Wc[14 * 128:] = np.eye(c.D, dtype=f32)
    Wcatd = np.ascontiguousarray(
        Wc.astype(FP8).reshape(c.KP, 128, c.D).transpose(1, 0, 2)
    ).reshape(128, c.KP * c.D)

    # session weight chunks: rr col layout = emb(0:128) img(128:1128)
    # txt(1128:1896) ind(1896) pad(1897:1920)
    W2 = np.zeros((c.NW, 128, c.D), f32)
    rows = np.arange(128)
    for i, m in enumerate(range(1, 9)):
        gl = m * 128 + rows
        fi = gl - 128
        val = np.where((gl >= 128) & (gl < 1128), 1.0, 0.0)
        W2[i] = np.asarray(img_W)[np.clip(fi, 0, c.IMG - 1)] * val[:, None]
    W2[8, 104] = np.asarray(img_b)
    for i, m in enumerate(range(8, 15)):
        gl = m * 128 + rows
        fi = gl - 1128
        val = np.where((gl >= 1128) & (gl < 1896), 1.0, 0.0)
        W2[9 + i] = np.asarray(txt_W)[np.clip(fi, 0, c.TXT - 1)] * val[:, None]
    W2[16, 104] = np.asarray(txt_b)
    Wsd = np.ascontiguousarray(
        W2.astype(BF16).transpose(1, 0, 2)).reshape(128, c.NW * c.D)

    def gather(tab, ids):
        r = tab[np.maximum(ids - 1, 0)]
        r[ids == 0] = 0.0
        return r

    in_maps = []
    for kk in range(c.NC):
        b0, b1 = kk * c.BS, (kk + 1) * c.BS
        # --- h0 projection inputs (pair-major entry order) ---
        iid = inputs[b0:b1].reshape(c.NPAIR, c.L2).ravel()
        A = np.zeros((c.KP * 128, c.NE), FP8)
        A[:c.IMG] = gather(img_table, iid).T.astype(FP8)
        A[c.IMG:c.IMG + c.TXT] = gather(txt_table, iid).T.astype(FP8)
        ge = gather(emb_table, iid) + bcomb
        ge[iid == 0] = 0.0
        A[14 * 128:] = ge.T.astype(FP8)
        A3 = A.reshape(c.KP, 128, c.NE)
        blocks = [np.ascontiguousarray(
            A3[:, :, r0:r0 + w].transpose(1, 0, 2)).reshape(128, c.KP * w)
            for r0, w in c.RCH]
        prTd = np.concatenate(blocks, axis=1)

        # --- session raw rows (b-major entry order, m-major chunks) ---
        tid = item[b0:b1].ravel()
        R = np.zeros((c.NE, c.RW), f32)
        re_ = gather(emb_table, tid) + bcomb
        re_[tid == 0] = 0.0
        R[:, 0:128] = re_
        R[:, 128:1128] = gather(img_table, tid)
        R[:, 1128:1896] = gather(txt_table, tid)
        R[:, 1896] = (tid > 0).astype(f32)
        rrd = np.ascontiguousarray(
            R.astype(FP8).reshape(c.KR, 128, c.MR, c.D).transpose(1, 2, 0, 3)
        ).reshape(128, c.MR * c.KR * c.D)

        mk = maskf[b0:b1]
        M = np.zeros((c.NE, c.BS), f32)
        M[np.arange(c.NE), np.arange(c.NE) // c.L] = mk.ravel()
        mkTd = np.ascontiguousarray(
            M.astype(FP8).reshape(c.KR, 128, c.BS).transpose(1, 0, 2)
        ).reshape(128, c.KR * c.BS)

        # --- hypergraph blocks ---
        Hk = Hs[b0:b1]
        Gd = np.zeros((c.L2, c.NPAIR, c.L2), f32)
        GTd = np.zeros((c.L2, c.NPAIR, c.L2), f32)
        for p in range(c.NPAIR):
            Gd[:c.L, p, :c.L] = Hk[2 * p]
            Gd[c.L:, p, c.L:] = Hk[2 * p + 1]
            GTd[:c.L, p, :c.L] = Hk[2 * p].T
            GTd[c.L:, p, c.L:] = Hk[2 * p + 1].T
        seldm = np.zeros((c.BS, c.NPAIR, c.L2), f32)
        for p in range(c.NPAIR):
            seldm[2 * p, p, :c.L] = 1.0
            seldm[2 * p + 1, p, c.L:] = 1.0

        in_maps.append({
            "prTd": prTd, "rrd": rrd, "mkTd": mkTd,
            "Wcatd": Wcatd, "Wsd": Wsd,
            "Gd": Gd.astype(FP8).reshape(c.L2, c.NPAIR * c.L2),
            "GTd": GTd.astype(FP8).reshape(c.L2, c.NPAIR * c.L2),
            "seld": seldm.astype(BF16).reshape(c.BS, c.NPAIR * c.L2),
            "mbld": np.ascontiguousarray(mk.T).astype(BF16),
            "gvW": np.asarray(gate_v_W).astype(f32),
            "gvB": np.asarray(gate_v_b).reshape(c.D, 1).astype(f32),
            "gtW": np.asarray(gate_t_W).astype(f32),
            "gtB": np.asarray(gate_t_b).reshape(c.D, 1).astype(f32),
            "q1W": np.asarray(qc_W1).astype(f32),
            "q1B": np.asarray(qc_b1).reshape(c.D, 1).astype(f32),
            "q2W": np.asarray(qc_W2).astype(f32),
        })
    return in_maps


def run(c: Cfg, trace=False, **inputs):
    nc = _get_program(c)
    in_maps = _prep_inputs(c, **{k: np.asarray(v) for k, v in inputs.items()})
    res = bass_utils.run_bass_kernel_spmd(
        nc, in_maps, core_ids=list(range(c.NC)), trace=trace)
    outs = []
    for r in res.results:
        o = np.asarray(r["outd"]).astype(np.float32)
        o = o.reshape(c.L2, c.NPAIR, c.D).transpose(1, 0, 2)
        outs.append(o.reshape(c.NPAIR, 2, c.L, c.D).reshape(c.BS, c.L, c.D))
    out = np.concatenate(outs, axis=0)
    return out.astype(np.float32), res


def kernel(**inputs):
    out, _ = run(REAL, trace=False, **inputs)
    return out
